# revision 3
# baseline (speedup 1.0000x reference)
"""Link-predictor GNN kernel for 8 TRN2 NeuronCores.

Strategy (per sharding hint): shard edges across 8 cores (data parallel),
replicate the bf16 node-embedding table + MLP weights on every core.

Edges are bucketed by (src_window, dst_window) where a window is 25000
table rows (4 windows cover 100000 nodes) so window-relative node ids fit
the int16 indices of the batched SWDGE dma_gather.

DMA-descriptor reduction via 512B paired descriptors: the SWDGE cost is
per-descriptor with a 2x penalty under 512 bytes, so a 256-element bf16
descriptor (elem_step=128: table rows u and u+1) costs the same as a
128-element one but carries two rows. Per bucket, edges whose SRC rows are
adjacent (u, u+1) are paired globally (path-greedy along each window),
leftovers are paired again by DST adjacency, and the rest stay solo:
~85% of edges land in pairs, cutting gather descriptors per edge from
2.0 to ~1.58. A paired chunk of n pairs (2n edge columns) issues one
elem=256 gather (n descriptors, lands [128, 2, n]: sub-block 0 = first
rows, sub-block 1 = second rows, both in X^T layout) plus one per-slot
elem=128 gather for the other side (2n descriptors). Pairs/solos are
dealt round-robin to cores so the shared static per-bucket capacities
are tight.

MLP per 512-edge tile: h = relu(W1s^T Xs + W1d^T Xd + b1) via 4 matmuls
accumulating in PSUM; relu of h-half-0 on ACT (bias fused), half-1 on DVE
(tensor_scalar add+max). Layer 2 contracts h against W2 using h-subtiles
as the stationary operand: 2 matmuls of N=1 per 128-edge subtile writing
one PSUM column; a whole chunk's logits accumulate into one PSUM tile so a
single sigmoid + one small DMA per chunk emits [128, cols] f32 results.
Host inverts the slot permutation. The 16-partition-wrapped index arrays
are loaded once and replicated to the 128 partitions the gather hardware
expects via 0/1-matmul broadcasts of the raw bf16 bit patterns.
"""

import sys

sys.path.insert(0, "/opt/trn_rl_repo")

import numpy as np
import ml_dtypes

from concourse import bacc, mybir, tile
from concourse.ap import AP
from concourse.bass_utils import run_bass_kernel_spmd

BF16 = ml_dtypes.bfloat16

N_NODES = 100000
D = 128
H = 256
E_TOTAL = 600000
NCORES = 8
WIN = 25000                      # table-row window (< 2^15 for int16 idx)
NBUCK = 16                       # 4 src windows x 4 dst windows
CHUNK = 3072                     # max edge columns per compute chunk
XBUFS = 4                        # gather buffer depth per side
HPBUFS = 2                       # PSUM h depth
L2LAG = 1                        # tiles of lag between L1 and L2 issue
HBUFS = 3                        # h sbuf tile depth
TAPER = 8000                     # trailing cols re-split into smaller chunks
TAPER_PIECE = 1024               # taper piece size (multiple of 256)
PLBUFS = 2                       # logits PSUM depth
PBBUFS = 2                       # idx-broadcast PSUM depth
OBUFS = 3                        # sigmoid output tile depth
MIDSPLIT_N, MIDSPLIT_D = 1, 4    # mid output-store point (fraction of plan)

# (sp_pairs, dp_pairs, solo_cols) per bucket for the canonical
# setup_inputs() edge set. kernel() recomputes these from its actual
# inputs; this default only serves _build_program() callers that have no
# inputs (e.g. a standalone TimelineSim of the program).
DEFAULT_CAPS = (
    (1664, 1664, 1664, 1664, 1664, 1664, 1664, 1664,
     1664, 1664, 1664, 1664, 1664, 1664, 1664, 1664),
    (256, 256, 256, 256, 256, 256, 256, 256,
     256, 256, 256, 256, 256, 256, 256, 256),
    (896, 896, 896, 896, 896, 896, 896, 896,
     896, 1024, 896, 896, 896, 896, 896, 896),
)

LAST_RESULTS = None
_NC_CACHE: dict = {}

# chunk kinds
K_SP = 0   # src side pair-gathered, dst per-slot
K_DP = 1   # dst side pair-gathered, src per-slot
K_SO = 2   # both sides per-slot


def _window(w):
    base = w * WIN
    return base, min(WIN, N_NODES - base)


def _chunk_plan(caps):
    """Cut bucket segments into chunks and pick a processing order.

    caps = (sp[16], dp[16], so[16]): sp/dp in PAIRS (cols = 2x), so in cols.
    Returns list of (bucket, kind, col_off, ncols, slot_base, idx_base) with
    slot_base/idx_base assigned in processing order. col_off is the column
    offset within the (bucket, kind) segment. Descriptor counts per chunk:
    SP/DP: 1.5*ncols; SO: 2*ncols.
    """
    spc, dpc, soc = caps
    segs = []
    for b in range(NBUCK):
        if spc[b]:
            segs.append((b, K_SP, 2 * spc[b]))
        if dpc[b]:
            segs.append((b, K_DP, 2 * dpc[b]))
        if soc[b]:
            segs.append((b, K_SO, soc[b]))
    chunks = []
    for b, kind, cols in segs:
        o = 0
        while o < cols:
            sz = min(CHUNK, cols - o)
            chunks.append((b, kind, o, sz))
            o += sz
    chunks.sort(key=lambda c: -c[3])
    bigs = [c for c in chunks if c[3] >= CHUNK]
    smalls = [c for c in chunks if c[3] < CHUNK]
    seq = []
    first = smalls.pop() if smalls else (bigs.pop() if bigs else None)
    if first:
        seq.append(first)
    seq.extend(bigs)
    seq.extend(smalls)
    # taper: re-split the trailing cols into small chunks so the drain after
    # the last big transfer is short
    tail = []
    acc = 0
    while seq and acc < TAPER and seq[-1][3] > TAPER_PIECE:
        b, kind, o, sz = seq.pop()
        for i in range(0, sz, TAPER_PIECE):
            tail.append((b, kind, o + i, min(TAPER_PIECE, sz - i)))
        acc += sz
    seq.extend(tail)
    plan = []
    sbase = 0
    ibase = 0
    for b, kind, o, sz in seq:
        nd = 2 * sz if kind == K_SO else (3 * sz) // 2
        plan.append((b, kind, o, sz, sbase, ibase))
        sbase += sz
        ibase += nd
    return plan


def _plan_sizes(caps):
    plan = _chunk_plan(caps)
    ep = sum(p[3] for p in plan)
    dtot = sum(2 * p[3] if p[1] == K_SO else (3 * p[3]) // 2 for p in plan)
    return plan, ep, dtot


def _build_program(caps=None):
    if caps is None:
        caps = DEFAULT_CAPS
    caps = tuple(tuple(int(x) for x in c) for c in caps)
    if caps in _NC_CACHE:
        return _NC_CACHE[caps]

    dt = mybir.dt
    AF = mybir.ActivationFunctionType
    ALU = mybir.AluOpType

    plan, EP, DTOT = _plan_sizes(caps)
    TOT = EP // 128
    CMAX = max(p[3] for p in plan)

    nc = bacc.Bacc(
        "TRN2",
        target_bir_lowering=False,
        debug=False,
        enable_asserts=False,
        num_devices=NCORES,
    )
    emd = nc.dram_tensor("emd", [N_NODES, D], dt.bfloat16, kind="ExternalInput")
    idx_d = nc.dram_tensor(
        "idx", [16, 128 + DTOT // 16], dt.int16, kind="ExternalInput"
    )
    w1_d = nc.dram_tensor("w1", [128, 512], dt.bfloat16, kind="ExternalInput")
    b1_d = nc.dram_tensor("b1", [128, 2], dt.float32, kind="ExternalInput")
    w2_d = nc.dram_tensor("w2", [128, 2], dt.bfloat16, kind="ExternalInput")
    b2_d = nc.dram_tensor("b2", [128, 1], dt.float32, kind="ExternalInput")
    out_d = nc.dram_tensor("out", [128, TOT], dt.float32, kind="ExternalOutput")

    with tile.TileContext(nc) as tc:
        with (
            tc.tile_pool(name="const", bufs=1) as cpool,
            tc.tile_pool(name="x", bufs=XBUFS) as xpool,
            tc.tile_pool(name="h", bufs=HBUFS) as hpool,
            tc.tile_pool(name="o", bufs=OBUFS) as opool,
            tc.tile_pool(name="ph", bufs=HPBUFS, space="PSUM") as php,
            tc.tile_pool(name="pl", bufs=PLBUFS, space="PSUM") as plp,
            tc.tile_pool(name="pb", bufs=PBBUFS, space="PSUM") as pbp,
        ):
            # one combined load: the 0/1 selection matrix (bf16-bitcast) in
            # cols 0:128, then all (16-partition-wrapped) gather indices.
            selidx_sb = cpool.tile([16, 128 + DTOT // 16], dt.int16)
            nc.sync.dma_start(selidx_sb[:, :], idx_d[:, :])
            sel_sb = selidx_sb[:, 0:128].bitcast(dt.bfloat16)
            idx16_sb = selidx_sb[:, 128:]
            osb = cpool.tile([128, TOT], dt.float32)
            w1_sb = cpool.tile([128, 512], dt.bfloat16)
            nc.scalar.dma_start(w1_sb[:, :], w1_d[:, :])
            b1_sb = cpool.tile([128, 2], dt.float32)
            nc.scalar.dma_start(b1_sb[:, :], b1_d[:, :])
            w2_sb = cpool.tile([128, 2], dt.bfloat16)
            nc.scalar.dma_start(w2_sb[:, :], w2_d[:, :])
            b2_sb = cpool.tile([128, 1], dt.float32)
            nc.scalar.dma_start(b2_sb[:, :], b2_d[:, :])

            # broadcast every chunk's indices to 128 partitions up front;
            # PE/DVE pipeline stays far ahead of the gathers consuming them
            midcol = 0
            sdis = []
            for k, (b, kind, o, sz, sbase, ibase) in enumerate(plan):
                nd = 2 * sz if kind == K_SO else (3 * sz) // 2
                c16 = ibase // 16
                ibx = pbp.tile([128, CMAX // 8], dt.float32, tag="ibx")
                nc.tensor.matmul(
                    ibx[:, 0 : nd // 16], lhsT=sel_sb,
                    rhs=idx16_sb[:, c16 : c16 + nd // 16].bitcast(dt.bfloat16),
                    start=True, stop=True,
                )
                sdi = cpool.tile([128, nd // 16], dt.int16, name=f"sdi{k}")
                nc.vector.tensor_copy(
                    out=sdi[:, :].bitcast(dt.bfloat16),
                    in_=ibx[:, 0 : nd // 16],
                )
                sdis.append(sdi)

            for k, (b, kind, o, sz, sbase, ibase) in enumerate(plan):
                ncols = sz // 128
                sb_, sl_ = _window(b >> 2)
                db_, dl_ = _window(b & 3)
                sdi = sdis[k]
                xs = xpool.tile([128, CMAX], dt.bfloat16, tag="xs")
                xd = xpool.tile([128, CMAX], dt.bfloat16, tag="xd")
                src_solo = emd[sb_ : sb_ + sl_, :]
                dst_solo = emd[db_ : db_ + dl_, :]
                if kind == K_SO:
                    si = sdi[:, 0 : sz // 16]
                    di = sdi[:, sz // 16 : sz // 8]
                    nc.gpsimd.dma_gather(
                        xs[:, 0:sz].unsqueeze(1), src_solo, si, sz, sz, D,
                        transpose=True, single_packet=False,
                    )
                    nc.gpsimd.dma_gather(
                        xd[:, 0:sz].unsqueeze(1), dst_solo, di, sz, sz, D,
                        transpose=True, single_packet=False,
                    )
                else:
                    n = sz // 2
                    pi = sdi[:, 0 : n // 16]
                    oi = sdi[:, n // 16 : (n + sz) // 16]
                    if kind == K_SP:
                        pbase, plen, ptile, otile, osolo = (
                            sb_, sl_, xs, xd, dst_solo)
                    else:
                        pbase, plen, ptile, otile, osolo = (
                            db_, dl_, xd, xs, src_solo)
                    base = emd[pbase : pbase + plen, :]
                    pview = AP(
                        base.tensor, base.offset, [[128, plen - 1], [1, 256]]
                    )
                    t = ptile[:, 0:sz]
                    out3 = AP(
                        t.tensor, t.offset, [[t.ap[0][0], 128], [n, 2], [1, n]]
                    )
                    nc.gpsimd.dma_gather(
                        out3, pview, pi, n, n, 256,
                        elem_step=128, transpose=True, single_packet=False,
                    )
                    nc.gpsimd.dma_gather(
                        otile[:, 0:sz].unsqueeze(1), osolo, oi, sz, sz, D,
                        transpose=True, single_packet=False,
                    )

                lg = plp.tile([128, CMAX // 128], dt.float32, tag="lg")
                ntile = (sz + 511) // 512
                # software pipeline: L2 of tile t issues L2LAG tiles late
                hq = []
                for t in range(ntile + L2LAG):
                    if t < ntile:
                        e0 = t * 512
                        n = min(512, sz - e0)
                        h0p = php.tile([128, 512], dt.float32, tag="h0p")
                        h1p = php.tile([128, 512], dt.float32, tag="h1p")
                        nc.tensor.matmul(
                            h0p[:, 0:n], lhsT=w1_sb[:, 0:128],
                            rhs=xs[:, e0 : e0 + n], start=True, stop=False,
                        )
                        nc.tensor.matmul(
                            h0p[:, 0:n], lhsT=w1_sb[:, 256:384],
                            rhs=xd[:, e0 : e0 + n], start=False, stop=True,
                        )
                        nc.tensor.matmul(
                            h1p[:, 0:n], lhsT=w1_sb[:, 128:256],
                            rhs=xs[:, e0 : e0 + n], start=True, stop=False,
                        )
                        nc.tensor.matmul(
                            h1p[:, 0:n], lhsT=w1_sb[:, 384:512],
                            rhs=xd[:, e0 : e0 + n], start=False, stop=True,
                        )
                        h0s = hpool.tile([128, 512], dt.bfloat16, tag="h0s")
                        h1s = hpool.tile([128, 512], dt.bfloat16, tag="h1s")
                        nc.scalar.activation(
                            h0s[:, 0:n], h0p[:, 0:n], AF.Relu, bias=b1_sb[:, 0:1]
                        )
                        nc.vector.tensor_scalar(
                            h1s[:, 0:n], h1p[:, 0:n],
                            b1_sb[:, 1:2], 0.0, ALU.add, ALU.max,
                        )
                        hq.append((t, n, h0s, h1s))
                    if t >= L2LAG:
                        pt, pn, p0, p1 = hq[t - L2LAG]
                        for s in range((pn + 127) // 128):
                            ns = min(128, pn - s * 128)
                            col = pt * 4 + s
                            nc.tensor.matmul(
                                lg[0:ns, col : col + 1],
                                lhsT=p0[:, s * 128 : s * 128 + ns],
                                rhs=w2_sb[:, 0:1], start=True, stop=False,
                            )
                            nc.tensor.matmul(
                                lg[0:ns, col : col + 1],
                                lhsT=p1[:, s * 128 : s * 128 + ns],
                                rhs=w2_sb[:, 1:2], start=False, stop=True,
                            )
                nc.scalar.activation(
                    osb[:, sbase // 128 : sbase // 128 + ncols],
                    lg[:, 0:ncols], AF.Sigmoid, bias=b2_sb[:, 0:1],
                )
                if k == len(plan) * MIDSPLIT_N // MIDSPLIT_D and k < len(plan) - 2:
                    midcol = (sbase + sz) // 128
                    nc.sync.dma_start(out_d[:, 0:midcol], osb[:, 0:midcol])
                if k == len(plan) - 2 and midcol < (sbase + sz) // 128:
                    nextcol = (sbase + sz) // 128
                    nc.sync.dma_start(
                        out_d[:, midcol:nextcol], osb[:, midcol:nextcol]
                    )
                    midcol = nextcol
                if k == len(plan) - 1 and midcol < TOT:
                    nc.sync.dma_start(
                        out_d[:, midcol:TOT], osb[:, midcol:TOT]
                    )

    nc.compile()
    _NC_CACHE[caps] = nc
    return nc


def _wrap_idx(vals):
    """int16 [n] -> [16, n//16] wrapped in 16 partitions."""
    n = vals.shape[0]
    return np.ascontiguousarray(vals.reshape(n // 16, 16).T)


def _path_greedy(pos_vals):
    """Pair refs at adjacent window-local positions (p, p+1), each ref used
    once, left-to-right greedy with carry (optimal for paths).

    pos_vals: int array of window-local positions. Returns (a_idx, b_idx):
    indices into pos_vals; ref a at position p pairs with ref b at p+1.
    """
    order = np.argsort(pos_vals, kind="stable")
    pv = pos_vals[order]
    cnt = np.bincount(pv, minlength=WIN)
    starts = np.zeros(WIN + 1, np.int64)
    np.cumsum(cnt, out=starts[1:])
    pairs_a = []
    pairs_b = []
    avail = 0
    prev_p = -2
    for p in np.nonzero(cnt)[0]:
        c = int(cnt[p])
        s = int(starts[p])
        if p == prev_p + 1 and avail > 0:
            t = min(avail, c)
            ps = int(starts[prev_p])
            pc = int(cnt[prev_p])
            pairs_a.append(order[ps + pc - avail : ps + pc - avail + t])
            pairs_b.append(order[s : s + t])
            avail = c - t
        else:
            avail = c
        prev_p = p
    if pairs_a:
        return np.concatenate(pairs_a), np.concatenate(pairs_b)
    return np.empty(0, np.int64), np.empty(0, np.int64)


def _global_plan(ei):
    """Bucket + pair all edges globally, deal to cores.

    Returns (caps, per_core) where caps = (sp[16], dp[16], so[16]) and
    per_core[c] = dict b -> (sp_pairs [p,2], dp_pairs [p,2], solo [s])
    holding GLOBAL edge ids (sp/dp rows are (edgeA, edgeB))."""
    src = ei[:, 0].astype(np.int64)
    dst = ei[:, 1].astype(np.int64)
    ws = src // WIN
    wd = dst // WIN
    bucket = ws * 4 + wd

    spc = [0] * NBUCK
    dpc = [0] * NBUCK
    soc = [0] * NBUCK
    per_core = [dict() for _ in range(NCORES)]
    for b in range(NBUCK):
        sel = np.nonzero(bucket == b)[0]
        # round 1: src adjacency
        a, bb = _path_greedy(src[sel] % WIN)
        paired = np.zeros(len(sel), bool)
        paired[a] = True
        paired[bb] = True
        sp = np.stack([sel[a], sel[bb]], axis=1) if len(a) else \
            np.empty((0, 2), np.int64)
        # round 2: dst adjacency among leftovers
        left = np.nonzero(~paired)[0]
        a2, b2 = _path_greedy(dst[sel[left]] % WIN)
        lp = np.zeros(len(left), bool)
        lp[a2] = True
        lp[b2] = True
        dp = np.stack([sel[left[a2]], sel[left[b2]]], axis=1) if len(a2) else \
            np.empty((0, 2), np.int64)
        so = sel[left[~lp]]

        # per-core shares; caps rounded DOWN to 128-pair granularity, excess
        # pairs demoted to solos per core; solo cap rounded UP to 128 cols.
        nsp = len(sp) // NCORES // 128 * 128
        ndp = len(dp) // NCORES // 128 * 128
        spc[b] = nsp
        dpc[b] = ndp
        max_solo = 0
        for c in range(NCORES):
            csp = sp[c::NCORES]
            cdp = dp[c::NCORES]
            cso = so[c::NCORES]
            demo = np.concatenate(
                [csp[nsp:].reshape(-1), cdp[ndp:].reshape(-1), cso]
            )
            per_core[c][b] = (csp[:nsp], cdp[:ndp], demo)
            max_solo = max(max_solo, len(demo))
        soc[b] = -(-max_solo // 128) * 128
    return (tuple(spc), tuple(dpc), tuple(soc)), per_core


def _prepare_core(core_items, plan, EP, DTOT, src, dst):
    """Build one core's wrapped idx array + slot->edge map for the shared
    chunk plan. core_items: dict b -> (sp, dp, solo) global-edge-id arrays."""
    idx = np.zeros((16, DTOT // 16), np.int16)
    edge_of_slot = np.full(EP, -1, np.int64)
    for b, kind, o, sz, sbase, ibase in plan:
        sp, dp, solo = core_items[b]
        sb_ = (b >> 2) * WIN
        db_ = (b & 3) * WIN
        c16 = ibase // 16
        if kind == K_SO:
            ids = solo[o : o + sz]
            m = len(ids)
            sc = np.zeros(sz, np.int16)
            dc = np.zeros(sz, np.int16)
            sc[:m] = (src[ids] - sb_).astype(np.int16)
            dc[:m] = (dst[ids] - db_).astype(np.int16)
            edge_of_slot[sbase : sbase + m] = ids
            idx[:, c16 : c16 + sz // 16] = _wrap_idx(sc)
            idx[:, c16 + sz // 16 : c16 + sz // 8] = _wrap_idx(dc)
        else:
            n = sz // 2
            po = o // 2
            pairs = (sp if kind == K_SP else dp)[po : po + n]
            m = len(pairs)
            ea = np.zeros(n, np.int64)
            eb = np.zeros(n, np.int64)
            if m:
                ea[:m] = pairs[:, 0]
                eb[:m] = pairs[:, 1]
                edge_of_slot[sbase : sbase + m] = pairs[:, 0]
                edge_of_slot[sbase + n : sbase + n + m] = pairs[:, 1]
            pc = np.zeros(n, np.int16)
            oc = np.zeros(sz, np.int16)
            if kind == K_SP:
                if m:
                    pc[:m] = (src[pairs[:, 0]] - sb_).astype(np.int16)
                    oc[:m] = (dst[pairs[:, 0]] - db_).astype(np.int16)
                    oc[n : n + m] = (dst[pairs[:, 1]] - db_).astype(np.int16)
            else:
                if m:
                    pc[:m] = (dst[pairs[:, 0]] - db_).astype(np.int16)
                    oc[:m] = (src[pairs[:, 0]] - sb_).astype(np.int16)
                    oc[n : n + m] = (src[pairs[:, 1]] - sb_).astype(np.int16)
            idx[:, c16 : c16 + n // 16] = _wrap_idx(pc)
            idx[:, c16 + n // 16 : c16 + n // 16 + sz // 16] = _wrap_idx(oc)
    return idx, edge_of_slot


def kernel(emd_all, edge_index, W1, b1, W2, b2):
    global LAST_RESULTS
    emd_bf = np.ascontiguousarray(np.asarray(emd_all, dtype=np.float32)).astype(BF16)
    ei = np.asarray(edge_index).astype(np.int64)
    W1 = np.asarray(W1, dtype=np.float32)
    W2 = np.asarray(W2, dtype=np.float32)
    b1 = np.asarray(b1, dtype=np.float32).reshape(-1)
    b2 = np.asarray(b2, dtype=np.float32).reshape(-1)
    src = ei[:, 0].astype(np.int64)
    dst = ei[:, 1].astype(np.int64)

    caps, per_core = _global_plan(ei)
    plan, EP, DTOT = _plan_sizes(caps)

    # lhsT blocks: [src->h0, src->h1, dst->h0, dst->h1]
    w1_arr = np.concatenate(
        [W1[:D, :D], W1[:D, D:], W1[D:, :D], W1[D:, D:]], axis=1
    ).astype(BF16)
    b1_arr = np.ascontiguousarray(np.stack([b1[:128], b1[128:]], axis=1))
    w2_arr = np.ascontiguousarray(np.stack([W2[:128, 0], W2[128:, 0]], axis=1)).astype(
        BF16
    )
    b2_arr = np.full((128, 1), b2[0], np.float32)
    sel_arr = np.zeros((16, 128), np.float32)
    sel_arr[np.arange(128) % 16, np.arange(128)] = 1.0
    sel_arr = sel_arr.astype(BF16).view(np.int16)

    in_maps = []
    unshard = []
    for c in range(NCORES):
        idx, edge_of_slot = _prepare_core(per_core[c], plan, EP, DTOT, src, dst)
        unshard.append(edge_of_slot)
        in_maps.append(
            {
                "emd": emd_bf,
                "idx": np.concatenate([sel_arr, idx], axis=1),
                "w1": w1_arr,
                "b1": b1_arr,
                "w2": w2_arr,
                "b2": b2_arr,
            }
        )

    nc = _build_program(caps)
    res = run_bass_kernel_spmd(nc, in_maps, core_ids=list(range(NCORES)))
    LAST_RESULTS = res

    y = np.empty((E_TOTAL,), np.float32)
    for c in range(NCORES):
        edge_of_slot = unshard[c]  # slot -> global edge id
        out = np.asarray(res.results[c]["out"], dtype=np.float32)  # [128, TOT]
        flat = out.T.reshape(-1)  # slot-ordered
        mask = edge_of_slot >= 0
        y[edge_of_slot[mask]] = flat[mask]
    return y.reshape(E_TOTAL, 1)


if __name__ == "__main__":
    rng = np.random.default_rng(0)
    emd = rng.standard_normal((N_NODES, D), dtype=np.float32)
    ei = rng.integers(0, N_NODES, size=(E_TOTAL, 2)).astype(np.int32)
    W1 = rng.standard_normal((2 * D, H), dtype=np.float32) / np.sqrt(2 * D)
    W2 = rng.standard_normal((H, 1), dtype=np.float32) / np.sqrt(H)
    out = kernel(emd, ei, W1, np.zeros(H, np.float32), W2, np.zeros(1, np.float32))
    print(out.shape, out[:4, 0])


# revision 5
# speedup vs baseline: 1.1584x; 1.1584x over previous
"""Link-predictor GNN kernel for 8 TRN2 NeuronCores.

Strategy (per sharding hint): shard edges across 8 cores (data parallel),
replicate the bf16 node-embedding table + MLP weights on every core.

Edges are bucketed by (src_window, dst_window) where a window is 25000
table rows (4 windows cover 100000 nodes) so window-relative node ids fit
the int16 indices of the batched SWDGE dma_gather.

DMA-descriptor reduction via 512B paired descriptors: the SWDGE cost is
per-descriptor with a 2x penalty under 512 bytes, so a 256-element bf16
descriptor (elem_step=128: table rows u and u+1) costs the same as a
128-element one but carries two rows. Per bucket, edges whose SRC rows are
adjacent (u, u+1) are paired globally (path-greedy along each window),
leftovers are paired again by DST adjacency, and the rest stay solo:
~85% of edges land in pairs, cutting gather descriptors per edge from
2.0 to ~1.6. A paired call of n pairs lands [128, 2, n] (sub-block 0 =
first rows, sub-block 1 = second rows, both in X^T layout); the other
side uses a per-slot elem=128 gather. Pairs/solos are dealt round-robin
to cores so the shared static per-bucket capacities are tight.

Each bucket's [SP pairs | solo | DP pairs] column layout is processed as
ONE chunk with 4 merged gather calls (src-pair, src-solo over solo+DP,
dst-solo over SP+solo, dst-pair), keeping the Pool engine's ~1us fixed
SWDGE overhead per call well below the DMA transfer time so descriptor
generation always runs ahead. The first and last buckets are split into
small pieces so the DMA pipeline fills fast and drains short.

MLP per 512-edge tile: h = relu(W1s^T Xs + W1d^T Xd + b1) via 4 matmuls
accumulating in PSUM; relu of h-half-0 on ACT (bias fused), half-1 on DVE
(tensor_scalar add+max). Layer 2 contracts h against W2 using h-subtiles
as the stationary operand: 2 matmuls of N=1 per 128-edge subtile writing
one PSUM column; a whole chunk's logits accumulate into one PSUM tile so a
single sigmoid + one small DMA per chunk emits [128, cols] f32 results.
Host inverts the slot permutation. The 16-partition-wrapped index arrays
are loaded once and replicated to the 128 partitions the gather hardware
expects via 0/1-matmul broadcasts of the raw bf16 bit patterns.
"""

import sys

sys.path.insert(0, "/opt/trn_rl_repo")

import numpy as np
import ml_dtypes

from concourse import bacc, mybir, tile
from concourse.ap import AP
from concourse.bass_utils import run_bass_kernel_spmd

BF16 = ml_dtypes.bfloat16

N_NODES = 100000
D = 128
H = 256
E_TOTAL = 600000
NCORES = 8
WIN = 25000                      # table-row window (< 2^15 for int16 idx)
NBUCK = 16                       # 4 src windows x 4 dst windows
XBUFS = 6                        # gather buffer depth per side
HPBUFS = 2                       # PSUM h depth
L2LAG = 1                        # tiles of lag between L1 and L2 issue
HBUFS = 3                        # h sbuf tile depth
PLBUFS = 2                       # logits PSUM depth
PBBUFS = 2                       # idx-broadcast PSUM depth
OBUFS = 3                        # sigmoid output tile depth
FILL_PIECES = (256, 1024)        # leading col-cuts of the first bucket
TAIL_PIECE = 1024                # trailing bucket split granularity
MIDSPLIT_N, MIDSPLIT_D = 1, 4    # mid output-store point (fraction of plan)

# (sp_pairs, dp_pairs, solo_cols) per bucket for the canonical
# setup_inputs() edge set. kernel() recomputes these from its actual
# inputs; this default only serves _build_program() callers that have no
# inputs (e.g. a standalone TimelineSim of the program).
DEFAULT_CAPS = (
    (1664, 1664, 1664, 1664, 1664, 1664, 1664, 1664,
     1664, 1664, 1664, 1664, 1664, 1664, 1664, 1664),
    (256, 256, 256, 256, 256, 256, 256, 256,
     256, 256, 256, 256, 256, 256, 256, 256),
    (1024, 1024, 1024, 1024, 1024, 1024, 1024, 1024,
     1024, 1024, 1024, 1024, 1024, 1024, 1024, 1024),
)  # matches _global_plan(setup_inputs()['edge_index'])

LAST_RESULTS = None
_NC_CACHE: dict = {}


def _window(w):
    base = w * WIN
    return base, min(WIN, N_NODES - base)


def _bucket_cols(caps, b):
    spc, dpc, soc = caps
    return 2 * spc[b] + soc[b] + 2 * dpc[b]


def _calls_for_range(caps, b, lo, hi):
    """Gather calls covering chunk-relative cols [lo, hi) of bucket b's
    [SP | solo | DP] layout. Returns [(side, paired, col_off, ncols)] with
    col_off relative to lo; adjacent same-(side,paired) solo ranges merged.
    Cuts must be 256-aligned so pair calls keep num_idxs % 128 == 0."""
    spc, dpc, soc = caps
    r1 = 2 * spc[b]
    r2 = r1 + soc[b]
    r3 = r2 + 2 * dpc[b]
    assert 0 <= lo < hi <= r3 and lo % 256 == 0 and (hi % 256 == 0 or hi == r3)
    calls = []  # src side
    a, bnd = max(lo, 0), min(hi, r1)
    if a < bnd:
        calls.append(("s", True, a - lo, bnd - a))
    a, bnd = max(lo, r1), min(hi, r3)
    if a < bnd:
        calls.append(("s", False, a - lo, bnd - a))
    a, bnd = max(lo, 0), min(hi, r2)
    if a < bnd:
        calls.append(("d", False, a - lo, bnd - a))
    a, bnd = max(lo, r2), min(hi, r3)
    if a < bnd:
        calls.append(("d", True, a - lo, bnd - a))
    return calls


def _chunk_plan(caps):
    """Ordered chunk list. Chunk = (bucket, lo, cols, slot_base, idx_base,
    calls) where calls = [(side, paired, col_off, ncols, idx_off)], idx_off
    in descriptors relative to idx_base."""
    pieces = []  # (bucket, lo, hi)
    for b in range(NBUCK):
        cols = _bucket_cols(caps, b)
        if b == 0:
            o = 0
            for fp in FILL_PIECES:
                if o + fp < cols:
                    pieces.append((b, o, o + fp))
                    o += fp
            if o < cols:
                pieces.append((b, o, cols))
        elif b == NBUCK - 1:
            for o in range(0, cols, TAIL_PIECE):
                pieces.append((b, o, min(o + TAIL_PIECE, cols)))
        else:
            pieces.append((b, 0, cols))
    plan = []
    sbase = 0
    ibase = 0
    for b, lo, hi in pieces:
        calls = []
        ioff = 0
        for side, paired, coff, ncols in _calls_for_range(caps, b, lo, hi):
            nd = ncols // 2 if paired else ncols
            calls.append((side, paired, coff, ncols, ioff))
            ioff += nd
        plan.append((b, lo, hi - lo, sbase, ibase, calls))
        sbase += hi - lo
        ibase += ioff
    return plan


def _plan_sizes(caps):
    plan = _chunk_plan(caps)
    ep = sum(p[2] for p in plan)
    last = plan[-1]
    dtot = last[4] + sum(
        (nc // 2 if paired else nc) for _, paired, _, nc, _ in last[5]
    )
    return plan, ep, dtot


def _build_program(caps=None):
    if caps is None:
        caps = DEFAULT_CAPS
    caps = tuple(tuple(int(x) for x in c) for c in caps)
    if caps in _NC_CACHE:
        return _NC_CACHE[caps]

    dt = mybir.dt
    AF = mybir.ActivationFunctionType
    ALU = mybir.AluOpType

    plan, EP, DTOT = _plan_sizes(caps)
    TOT = EP // 128
    CMAX = max(p[2] for p in plan)
    NDMAX = max(
        sum(nc // 2 if paired else nc for _, paired, _, nc, _ in p[5])
        for p in plan
    )

    nc = bacc.Bacc(
        "TRN2",
        target_bir_lowering=False,
        debug=False,
        enable_asserts=False,
        num_devices=NCORES,
    )
    emd = nc.dram_tensor("emd", [N_NODES, D], dt.bfloat16, kind="ExternalInput")
    idx_d = nc.dram_tensor(
        "idx", [16, 128 + DTOT // 16], dt.int16, kind="ExternalInput"
    )
    w1_d = nc.dram_tensor("w1", [128, 512], dt.bfloat16, kind="ExternalInput")
    b1_d = nc.dram_tensor("b1", [128, 2], dt.float32, kind="ExternalInput")
    w2_d = nc.dram_tensor("w2", [128, 2], dt.bfloat16, kind="ExternalInput")
    b2_d = nc.dram_tensor("b2", [128, 1], dt.float32, kind="ExternalInput")
    out_d = nc.dram_tensor("out", [128, TOT], dt.float32, kind="ExternalOutput")

    with tile.TileContext(nc) as tc:
        with (
            tc.tile_pool(name="const", bufs=1) as cpool,
            tc.tile_pool(name="x", bufs=XBUFS) as xpool,
            tc.tile_pool(name="h", bufs=HBUFS) as hpool,
            tc.tile_pool(name="o", bufs=OBUFS) as opool,
            tc.tile_pool(name="ph", bufs=HPBUFS, space="PSUM") as php,
            tc.tile_pool(name="pl", bufs=PLBUFS, space="PSUM") as plp,
            tc.tile_pool(name="pb", bufs=PBBUFS, space="PSUM") as pbp,
        ):
            # one combined load: the 0/1 selection matrix (bf16-bitcast) in
            # cols 0:128, then all (16-partition-wrapped) gather indices.
            selidx_sb = cpool.tile([16, 128 + DTOT // 16], dt.int16)
            nc.sync.dma_start(selidx_sb[:, :], idx_d[:, :])
            sel_sb = selidx_sb[:, 0:128].bitcast(dt.bfloat16)
            idx16_sb = selidx_sb[:, 128:]
            osb = cpool.tile([128, TOT], dt.float32)
            w1_sb = cpool.tile([128, 512], dt.bfloat16)
            nc.scalar.dma_start(w1_sb[:, :], w1_d[:, :])
            b1_sb = cpool.tile([128, 2], dt.float32)
            nc.scalar.dma_start(b1_sb[:, :], b1_d[:, :])
            w2_sb = cpool.tile([128, 2], dt.bfloat16)
            nc.scalar.dma_start(w2_sb[:, :], w2_d[:, :])
            b2_sb = cpool.tile([128, 1], dt.float32)
            nc.scalar.dma_start(b2_sb[:, :], b2_d[:, :])

            # broadcast every chunk's indices to 128 partitions up front;
            # PE/DVE pipeline stays far ahead of the gathers consuming them
            midcol = 0
            sdis = []
            for k, (b, lo, cols, sbase, ibase, calls) in enumerate(plan):
                nd = sum(c // 2 if p else c for _, p, _, c, _ in calls)
                c16 = ibase // 16
                ibx = pbp.tile([128, NDMAX // 16], dt.float32, tag="ibx")
                nc.tensor.matmul(
                    ibx[:, 0 : nd // 16], lhsT=sel_sb,
                    rhs=idx16_sb[:, c16 : c16 + nd // 16].bitcast(dt.bfloat16),
                    start=True, stop=True,
                )
                sdi = cpool.tile([128, nd // 16], dt.int16, name=f"sdi{k}")
                nc.vector.tensor_copy(
                    out=sdi[:, :].bitcast(dt.bfloat16),
                    in_=ibx[:, 0 : nd // 16],
                )
                sdis.append(sdi)

            for k, (b, lo, cols, sbase, ibase, calls) in enumerate(plan):
                ncols128 = cols // 128
                sb_, sl_ = _window(b >> 2)
                db_, dl_ = _window(b & 3)
                sdi = sdis[k]
                xs = xpool.tile([128, CMAX], dt.bfloat16, tag="xs")
                xd = xpool.tile([128, CMAX], dt.bfloat16, tag="xd")
                for side, paired, coff, cn, ioff in calls:
                    tile_, wb, wl = (
                        (xs, sb_, sl_) if side == "s" else (xd, db_, dl_)
                    )
                    if paired:
                        n = cn // 2
                        pi = sdi[:, ioff // 16 : (ioff + n) // 16]
                        base = emd[wb : wb + wl, :]
                        pview = AP(
                            base.tensor, base.offset,
                            [[128, wl - 1], [1, 256]],
                        )
                        t = tile_[:, coff : coff + cn]
                        out3 = AP(
                            t.tensor, t.offset,
                            [[t.ap[0][0], 128], [n, 2], [1, n]],
                        )
                        nc.gpsimd.dma_gather(
                            out3, pview, pi, n, n, 256,
                            elem_step=128, transpose=True, single_packet=False,
                        )
                    else:
                        si = sdi[:, ioff // 16 : (ioff + cn) // 16]
                        nc.gpsimd.dma_gather(
                            tile_[:, coff : coff + cn].unsqueeze(1),
                            emd[wb : wb + wl, :], si, cn, cn, D,
                            transpose=True, single_packet=False,
                        )

                lg = plp.tile([128, CMAX // 128], dt.float32, tag="lg")
                ntile = (cols + 511) // 512
                # software pipeline: L2 of tile t issues L2LAG tiles late
                hq = []
                for t in range(ntile + L2LAG):
                    if t < ntile:
                        e0 = t * 512
                        n = min(512, cols - e0)
                        h0p = php.tile([128, 512], dt.float32, tag="h0p")
                        h1p = php.tile([128, 512], dt.float32, tag="h1p")
                        nc.tensor.matmul(
                            h0p[:, 0:n], lhsT=w1_sb[:, 0:128],
                            rhs=xs[:, e0 : e0 + n], start=True, stop=False,
                        )
                        nc.tensor.matmul(
                            h0p[:, 0:n], lhsT=w1_sb[:, 256:384],
                            rhs=xd[:, e0 : e0 + n], start=False, stop=True,
                        )
                        nc.tensor.matmul(
                            h1p[:, 0:n], lhsT=w1_sb[:, 128:256],
                            rhs=xs[:, e0 : e0 + n], start=True, stop=False,
                        )
                        nc.tensor.matmul(
                            h1p[:, 0:n], lhsT=w1_sb[:, 384:512],
                            rhs=xd[:, e0 : e0 + n], start=False, stop=True,
                        )
                        h0s = hpool.tile([128, 512], dt.bfloat16, tag="h0s")
                        h1s = hpool.tile([128, 512], dt.bfloat16, tag="h1s")
                        nc.scalar.activation(
                            h0s[:, 0:n], h0p[:, 0:n], AF.Relu, bias=b1_sb[:, 0:1]
                        )
                        nc.vector.tensor_scalar(
                            h1s[:, 0:n], h1p[:, 0:n],
                            b1_sb[:, 1:2], 0.0, ALU.add, ALU.max,
                        )
                        hq.append((t, n, h0s, h1s))
                    if t >= L2LAG:
                        pt, pn, p0, p1 = hq[t - L2LAG]
                        for s in range((pn + 127) // 128):
                            ns = min(128, pn - s * 128)
                            col = pt * 4 + s
                            nc.tensor.matmul(
                                lg[0:ns, col : col + 1],
                                lhsT=p0[:, s * 128 : s * 128 + ns],
                                rhs=w2_sb[:, 0:1], start=True, stop=False,
                            )
                            nc.tensor.matmul(
                                lg[0:ns, col : col + 1],
                                lhsT=p1[:, s * 128 : s * 128 + ns],
                                rhs=w2_sb[:, 1:2], start=False, stop=True,
                            )
                nc.scalar.activation(
                    osb[:, sbase // 128 : sbase // 128 + ncols128],
                    lg[:, 0:ncols128], AF.Sigmoid, bias=b2_sb[:, 0:1],
                )
                if k == len(plan) * MIDSPLIT_N // MIDSPLIT_D and k < len(plan) - 2:
                    midcol = (sbase + cols) // 128
                    nc.sync.dma_start(out_d[:, 0:midcol], osb[:, 0:midcol])
                if k == len(plan) - 2 and midcol < (sbase + cols) // 128:
                    nextcol = (sbase + cols) // 128
                    nc.sync.dma_start(
                        out_d[:, midcol:nextcol], osb[:, midcol:nextcol]
                    )
                    midcol = nextcol
                if k == len(plan) - 1 and midcol < TOT:
                    nc.sync.dma_start(
                        out_d[:, midcol:TOT], osb[:, midcol:TOT]
                    )

    nc.compile()
    _NC_CACHE[caps] = nc
    return nc


def _wrap_idx(vals):
    """int16 [n] -> [16, n//16] wrapped in 16 partitions."""
    n = vals.shape[0]
    return np.ascontiguousarray(vals.reshape(n // 16, 16).T)


def _path_greedy(pos_vals):
    """Pair refs at adjacent window-local positions (p, p+1), each ref used
    once, left-to-right greedy with carry (optimal for paths).

    pos_vals: int array of window-local positions. Returns (a_idx, b_idx):
    indices into pos_vals; ref a at position p pairs with ref b at p+1.
    """
    order = np.argsort(pos_vals, kind="stable")
    pv = pos_vals[order]
    cnt = np.bincount(pv, minlength=WIN)
    starts = np.zeros(WIN + 1, np.int64)
    np.cumsum(cnt, out=starts[1:])
    pairs_a = []
    pairs_b = []
    avail = 0
    prev_p = -2
    for p in np.nonzero(cnt)[0]:
        c = int(cnt[p])
        s = int(starts[p])
        if p == prev_p + 1 and avail > 0:
            t = min(avail, c)
            ps = int(starts[prev_p])
            pc = int(cnt[prev_p])
            pairs_a.append(order[ps + pc - avail : ps + pc - avail + t])
            pairs_b.append(order[s : s + t])
            avail = c - t
        else:
            avail = c
        prev_p = p
    if pairs_a:
        return np.concatenate(pairs_a), np.concatenate(pairs_b)
    return np.empty(0, np.int64), np.empty(0, np.int64)


def _global_plan(ei):
    """Bucket + pair all edges globally, deal to cores.

    Returns (caps, per_core) where caps = (sp[16], dp[16], so[16]) (sp/dp in
    pairs, so in cols) and per_core[c][b] = (sp_pairs [p,2], dp_pairs [p,2],
    solo [s]) holding GLOBAL edge ids."""
    src = ei[:, 0].astype(np.int64)
    dst = ei[:, 1].astype(np.int64)
    bucket = (src // WIN) * 4 + dst // WIN

    spc = [0] * NBUCK
    dpc = [0] * NBUCK
    soc = [0] * NBUCK
    per_core = [dict() for _ in range(NCORES)]
    for b in range(NBUCK):
        sel = np.nonzero(bucket == b)[0]
        a, bb = _path_greedy(src[sel] % WIN)
        paired = np.zeros(len(sel), bool)
        paired[a] = True
        paired[bb] = True
        sp = np.stack([sel[a], sel[bb]], axis=1) if len(a) else \
            np.empty((0, 2), np.int64)
        left = np.nonzero(~paired)[0]
        a2, b2 = _path_greedy(dst[sel[left]] % WIN)
        lp = np.zeros(len(left), bool)
        lp[a2] = True
        lp[b2] = True
        dp = np.stack([sel[left[a2]], sel[left[b2]]], axis=1) if len(a2) else \
            np.empty((0, 2), np.int64)
        so = sel[left[~lp]]

        # per-core shares; pair caps rounded DOWN to 128-pair granularity,
        # excess pairs demoted to solos per core; solo cap rounded UP to 256
        # cols so every region boundary stays 256-aligned.
        nsp = len(sp) // NCORES // 128 * 128
        ndp = len(dp) // NCORES // 128 * 128
        spc[b] = nsp
        dpc[b] = ndp
        max_solo = 0
        for c in range(NCORES):
            csp = sp[c::NCORES]
            cdp = dp[c::NCORES]
            cso = so[c::NCORES]
            demo = np.concatenate(
                [csp[nsp:].reshape(-1), cdp[ndp:].reshape(-1), cso]
            )
            per_core[c][b] = (csp[:nsp], cdp[:ndp], demo)
            max_solo = max(max_solo, len(demo))
        soc[b] = -(-max_solo // 256) * 256
    return (tuple(spc), tuple(dpc), tuple(soc)), per_core


def _prepare_core(core_items, caps, plan, EP, DTOT, src, dst):
    """Build one core's wrapped idx array + slot->edge map for the shared
    chunk plan. core_items: dict b -> (sp, dp, solo) global-edge-id arrays."""
    spc, dpc, soc = caps
    idx = np.zeros((16, DTOT // 16), np.int16)
    edge_of_slot = np.full(EP, -1, np.int64)

    # per-bucket col -> edge map (built once per bucket)
    col_edge = {}
    for b in range(NBUCK):
        sp, dp, solo = core_items[b]
        cols = _bucket_cols(caps, b)
        ce = np.full(cols, -1, np.int64)
        r1 = 2 * spc[b]
        r2 = r1 + soc[b]
        # SP region pair p -> cols (2-aligned within each CALL, but the call
        # layout depends on the chunk split; store pairs positionally and let
        # the per-call loop place them)
        col_edge[b] = ce  # filled below per chunk for pair regions
        ce[r1 : r1 + len(solo)] = solo

    for b, lo, cols, sbase, ibase, calls in plan:
        sp, dp, solo = core_items[b]
        sb_ = (b >> 2) * WIN
        db_ = (b & 3) * WIN
        r1 = 2 * spc[b]
        r2 = r1 + soc[b]
        ce = col_edge[b]
        # place pair edges for pair calls of this chunk (sub-block layout is
        # per-call: first rows at cols [coff, coff+n), second rows at
        # [coff+n, coff+2n))
        for side, paired, coff, cn, ioff in calls:
            if not paired:
                continue
            n = cn // 2
            glo = lo + coff
            pairs = sp if side == "s" else dp
            p0 = (glo - (0 if side == "s" else r2)) // 2
            pc_ = pairs[p0 : p0 + n]
            m = len(pc_)
            if m:
                ce[glo : glo + m] = pc_[:, 0]
                ce[glo + n : glo + n + m] = pc_[:, 1]
        # now write idx values for every call
        for side, paired, coff, cn, ioff in calls:
            tab = src if side == "s" else dst
            base_ = sb_ if side == "s" else db_
            if paired:
                n = cn // 2
                e = ce[lo + coff : lo + coff + n]
                v = np.zeros(n, np.int16)
                m = e >= 0
                v[m] = (tab[e[m]] - base_).astype(np.int16)
                idx[:, (ibase + ioff) // 16 : (ibase + ioff + n) // 16] = \
                    _wrap_idx(v)
            else:
                e = ce[lo + coff : lo + coff + cn]
                v = np.zeros(cn, np.int16)
                m = e >= 0
                v[m] = (tab[e[m]] - base_).astype(np.int16)
                idx[:, (ibase + ioff) // 16 : (ibase + ioff + cn) // 16] = \
                    _wrap_idx(v)
        edge_of_slot[sbase : sbase + cols] = ce[lo : lo + cols]
    return idx, edge_of_slot


def kernel(emd_all, edge_index, W1, b1, W2, b2):
    global LAST_RESULTS
    emd_bf = np.ascontiguousarray(np.asarray(emd_all, dtype=np.float32)).astype(BF16)
    ei = np.asarray(edge_index).astype(np.int64)
    W1 = np.asarray(W1, dtype=np.float32)
    W2 = np.asarray(W2, dtype=np.float32)
    b1 = np.asarray(b1, dtype=np.float32).reshape(-1)
    b2 = np.asarray(b2, dtype=np.float32).reshape(-1)
    src = ei[:, 0].astype(np.int64)
    dst = ei[:, 1].astype(np.int64)

    caps, per_core = _global_plan(ei)
    plan, EP, DTOT = _plan_sizes(caps)

    # lhsT blocks: [src->h0, src->h1, dst->h0, dst->h1]
    w1_arr = np.concatenate(
        [W1[:D, :D], W1[:D, D:], W1[D:, :D], W1[D:, D:]], axis=1
    ).astype(BF16)
    b1_arr = np.ascontiguousarray(np.stack([b1[:128], b1[128:]], axis=1))
    w2_arr = np.ascontiguousarray(np.stack([W2[:128, 0], W2[128:, 0]], axis=1)).astype(
        BF16
    )
    b2_arr = np.full((128, 1), b2[0], np.float32)
    sel_arr = np.zeros((16, 128), np.float32)
    sel_arr[np.arange(128) % 16, np.arange(128)] = 1.0
    sel_arr = sel_arr.astype(BF16).view(np.int16)

    in_maps = []
    unshard = []
    for c in range(NCORES):
        idx, edge_of_slot = _prepare_core(
            per_core[c], caps, plan, EP, DTOT, src, dst
        )
        unshard.append(edge_of_slot)
        in_maps.append(
            {
                "emd": emd_bf,
                "idx": np.concatenate([sel_arr, idx], axis=1),
                "w1": w1_arr,
                "b1": b1_arr,
                "w2": w2_arr,
                "b2": b2_arr,
            }
        )

    nc = _build_program(caps)
    res = run_bass_kernel_spmd(nc, in_maps, core_ids=list(range(NCORES)))
    LAST_RESULTS = res

    y = np.empty((E_TOTAL,), np.float32)
    for c in range(NCORES):
        edge_of_slot = unshard[c]  # slot -> global edge id
        out = np.asarray(res.results[c]["out"], dtype=np.float32)  # [128, TOT]
        flat = out.T.reshape(-1)  # slot-ordered
        mask = edge_of_slot >= 0
        y[edge_of_slot[mask]] = flat[mask]
    return y.reshape(E_TOTAL, 1)


if __name__ == "__main__":
    rng = np.random.default_rng(0)
    emd = rng.standard_normal((N_NODES, D), dtype=np.float32)
    ei = rng.integers(0, N_NODES, size=(E_TOTAL, 2)).astype(np.int32)
    W1 = rng.standard_normal((2 * D, H), dtype=np.float32) / np.sqrt(2 * D)
    W2 = rng.standard_normal((H, 1), dtype=np.float32) / np.sqrt(H)
    out = kernel(emd, ei, W1, np.zeros(H, np.float32), W2, np.zeros(1, np.float32))
    print(out.shape, out[:4, 0])


# revision 8
# speedup vs baseline: 1.1890x; 1.0264x over previous
"""Link-predictor GNN kernel for 8 TRN2 NeuronCores.

Strategy (per sharding hint): shard edges across 8 cores (data parallel),
replicate the bf16 node-embedding table + MLP weights on every core.

Edges are bucketed by (src_window, dst_window) where a window is 25000
table rows (4 windows cover 100000 nodes) so window-relative node ids fit
the int16 indices of the batched SWDGE dma_gather.

DMA-descriptor reduction via 512B paired descriptors: the SWDGE cost is
per-descriptor with a 2x penalty under 512 bytes, so a 256-element bf16
descriptor (elem_step=128: table rows u and u+1) costs the same as a
128-element one but carries two rows. Per bucket, edges whose SRC rows are
adjacent (u, u+1) are paired globally (path-greedy along each window),
leftovers are paired again by DST adjacency, and the rest stay solo:
~85% of edges land in pairs, cutting gather descriptors per edge from
2.0 to ~1.6. A paired call of n pairs lands [128, 2, n] (sub-block 0 =
first rows, sub-block 1 = second rows, both in X^T layout); the other
side uses a per-slot elem=128 gather. Pairs/solos are dealt round-robin
to cores so the shared static per-bucket capacities are tight.

Each bucket's [SP pairs | solo | DP pairs] column layout is processed as
ONE chunk with 4 merged gather calls (src-pair, src-solo over solo+DP,
dst-solo over SP+solo, dst-pair), keeping the Pool engine's ~1us fixed
SWDGE overhead per call well below the DMA transfer time so descriptor
generation always runs ahead. The first and last buckets are split into
small pieces so the DMA pipeline fills fast and drains short.

MLP per 512-edge tile: h = relu(W1s^T Xs + W1d^T Xd + b1) via 4 matmuls
accumulating in PSUM; relu of h-half-0 on ACT (bias fused), half-1 on DVE
(tensor_scalar add+max). Layer 2 contracts h against W2 using h-subtiles
as the stationary operand: 2 matmuls of N=1 per 128-edge subtile writing
one PSUM column; a whole chunk's logits accumulate into one PSUM tile so a
single sigmoid + one small DMA per chunk emits [128, cols] f32 results.
Host inverts the slot permutation. The 16-partition-wrapped index arrays
are loaded once and replicated to the 128 partitions the gather hardware
expects via 0/1-matmul broadcasts of the raw bf16 bit patterns.
"""

import sys

sys.path.insert(0, "/opt/trn_rl_repo")

import numpy as np
import ml_dtypes

from concourse import bacc, mybir, tile
from concourse.ap import AP
from concourse.bass_utils import run_bass_kernel_spmd

BF16 = ml_dtypes.bfloat16

N_NODES = 100000
D = 128
H = 256
E_TOTAL = 600000
NCORES = 8
WIN = 25000                      # table-row window (< 2^15 for int16 idx)
NBUCK = 16                       # 4 src windows x 4 dst windows
XBUFS = 6                        # gather buffer depth per side
HPBUFS = 2                       # PSUM h depth
L2LAG = 1                        # tiles of lag between L1 and L2 issue
HBUFS = 3                        # h sbuf tile depth
PLBUFS = 2                       # logits PSUM depth
PBBUFS = 2                       # idx-broadcast PSUM depth
OBUFS = 3                        # sigmoid output tile depth
FILL_PIECES = (256,)             # leading col-cuts of the first bucket
TAIL_PIECE = 1024                # trailing bucket split granularity
MIDSPLIT_N, MIDSPLIT_D = 1, 4    # mid output-store point (fraction of plan)

# (sp_pairs, dp_pairs, solo_cols) per bucket for the canonical
# setup_inputs() edge set. kernel() recomputes these from its actual
# inputs; this default only serves _build_program() callers that have no
# inputs (e.g. a standalone TimelineSim of the program).
DEFAULT_CAPS = (
    (1664, 1664, 1664, 1664, 1664, 1664, 1664, 1664,
     1664, 1664, 1664, 1664, 1664, 1664, 1664, 1664),
    (256, 256, 256, 256, 256, 256, 256, 256,
     256, 256, 256, 256, 256, 256, 256, 256),
    (1024, 896, 896, 1024, 896, 896, 896, 896,
     896, 896, 896, 896, 896, 896, 896, 1024),
)  # matches _global_plan(setup_inputs()['edge_index'])

LAST_RESULTS = None
_NC_CACHE: dict = {}


def _window(w):
    base = w * WIN
    return base, min(WIN, N_NODES - base)


def _bucket_cols(caps, b):
    spc, dpc, soc = caps
    return 2 * spc[b] + soc[b] + 2 * dpc[b]


def _calls_for_range(caps, b, lo, hi):
    """Gather calls covering chunk-relative cols [lo, hi) of bucket b's
    [SP | solo | DP] layout. Returns [(side, paired, col_off, ncols)] with
    col_off relative to lo; adjacent same-(side,paired) solo ranges merged.
    Cuts must be 256-aligned so pair calls keep num_idxs % 128 == 0."""
    spc, dpc, soc = caps
    r1 = 2 * spc[b]
    r2 = r1 + soc[b]
    r3 = r2 + 2 * dpc[b]
    assert 0 <= lo < hi <= r3 and lo % 256 == 0 and (hi % 256 == 0 or hi == r3)
    calls = []  # src side
    a, bnd = max(lo, 0), min(hi, r1)
    if a < bnd:
        calls.append(("s", True, a - lo, bnd - a))
    a, bnd = max(lo, r1), min(hi, r3)
    if a < bnd:
        calls.append(("s", False, a - lo, bnd - a))
    a, bnd = max(lo, 0), min(hi, r2)
    if a < bnd:
        calls.append(("d", False, a - lo, bnd - a))
    a, bnd = max(lo, r2), min(hi, r3)
    if a < bnd:
        calls.append(("d", True, a - lo, bnd - a))
    return calls


def _chunk_plan(caps):
    """Ordered chunk list. Chunk = (bucket, lo, cols, slot_base, idx_base,
    calls) where calls = [(side, paired, col_off, ncols, idx_off)], idx_off
    in descriptors relative to idx_base."""
    pieces = []  # (bucket, lo, hi)
    for b in range(NBUCK):
        cols = _bucket_cols(caps, b)
        if b == 0:
            o = 0
            for fp in FILL_PIECES:
                if o + fp < cols:
                    pieces.append((b, o, o + fp))
                    o += fp
            if o < cols:
                pieces.append((b, o, cols))
        elif b == NBUCK - 1:
            for o in range(0, cols, TAIL_PIECE):
                pieces.append((b, o, min(o + TAIL_PIECE, cols)))
        else:
            pieces.append((b, 0, cols))
    plan = []
    sbase = 0
    ibase = 0
    for b, lo, hi in pieces:
        calls = []
        ioff = 0
        for side, paired, coff, ncols in _calls_for_range(caps, b, lo, hi):
            nd = ncols // 2 if paired else ncols
            calls.append((side, paired, coff, ncols, ioff))
            ioff += nd
        plan.append((b, lo, hi - lo, sbase, ibase, calls))
        sbase += hi - lo
        ibase += ioff
    return plan


def _plan_sizes(caps):
    plan = _chunk_plan(caps)
    ep = sum(p[2] for p in plan)
    last = plan[-1]
    dtot = last[4] + sum(
        (nc // 2 if paired else nc) for _, paired, _, nc, _ in last[5]
    )
    return plan, ep, dtot


def _build_program(caps=None):
    if caps is None:
        caps = DEFAULT_CAPS
    caps = tuple(tuple(int(x) for x in c) for c in caps)
    if caps in _NC_CACHE:
        return _NC_CACHE[caps]

    dt = mybir.dt
    AF = mybir.ActivationFunctionType
    ALU = mybir.AluOpType

    plan, EP, DTOT = _plan_sizes(caps)
    TOT = EP // 128
    CMAX = max(p[2] for p in plan)
    NDMAX = max(
        sum(nc // 2 if paired else nc for _, paired, _, nc, _ in p[5])
        for p in plan
    )

    nc = bacc.Bacc(
        "TRN2",
        target_bir_lowering=False,
        debug=False,
        enable_asserts=False,
        num_devices=NCORES,
    )
    emd = nc.dram_tensor("emd", [N_NODES, D], dt.bfloat16, kind="ExternalInput")
    idx_d = nc.dram_tensor(
        "idx", [16, 128 + DTOT // 16], dt.int16, kind="ExternalInput"
    )
    w1_d = nc.dram_tensor("w1", [128, 512], dt.bfloat16, kind="ExternalInput")
    b1_d = nc.dram_tensor("b1", [128, 2], dt.float32, kind="ExternalInput")
    w2_d = nc.dram_tensor("w2", [128, 2], dt.bfloat16, kind="ExternalInput")
    b2_d = nc.dram_tensor("b2", [128, 1], dt.float32, kind="ExternalInput")
    out_d = nc.dram_tensor("out", [128, TOT], dt.float32, kind="ExternalOutput")

    with tile.TileContext(nc) as tc:
        with (
            tc.tile_pool(name="const", bufs=1) as cpool,
            tc.tile_pool(name="x", bufs=XBUFS) as xpool,
            tc.tile_pool(name="h", bufs=HBUFS) as hpool,
            tc.tile_pool(name="o", bufs=OBUFS) as opool,
            tc.tile_pool(name="ph", bufs=HPBUFS, space="PSUM") as php,
            tc.tile_pool(name="pl", bufs=PLBUFS, space="PSUM") as plp,
            tc.tile_pool(name="pb", bufs=PBBUFS, space="PSUM") as pbp,
        ):
            # one combined load: the 0/1 selection matrix (bf16-bitcast) in
            # cols 0:128, then all (16-partition-wrapped) gather indices.
            selidx_sb = cpool.tile([16, 128 + DTOT // 16], dt.int16)
            nc.sync.dma_start(selidx_sb[:, :], idx_d[:, :])
            sel_sb = selidx_sb[:, 0:128].bitcast(dt.bfloat16)
            idx16_sb = selidx_sb[:, 128:]
            osb = cpool.tile([128, TOT], dt.float32)
            w1_sb = cpool.tile([128, 512], dt.bfloat16)
            nc.scalar.dma_start(w1_sb[:, :], w1_d[:, :])
            b1_sb = cpool.tile([128, 2], dt.float32)
            nc.scalar.dma_start(b1_sb[:, :], b1_d[:, :])
            w2_sb = cpool.tile([128, 2], dt.bfloat16)
            nc.scalar.dma_start(w2_sb[:, :], w2_d[:, :])
            b2_sb = cpool.tile([128, 1], dt.float32)
            nc.scalar.dma_start(b2_sb[:, :], b2_d[:, :])

            # broadcast every chunk's indices to 128 partitions up front;
            # PE/DVE pipeline stays far ahead of the gathers consuming them
            midcol = 0
            sdis = []
            for k, (b, lo, cols, sbase, ibase, calls) in enumerate(plan):
                nd = sum(c // 2 if p else c for _, p, _, c, _ in calls)
                c16 = ibase // 16
                ibx = pbp.tile([128, NDMAX // 16], dt.float32, tag="ibx")
                nc.tensor.matmul(
                    ibx[:, 0 : nd // 16], lhsT=sel_sb,
                    rhs=idx16_sb[:, c16 : c16 + nd // 16].bitcast(dt.bfloat16),
                    start=True, stop=True,
                )
                sdi = cpool.tile([128, nd // 16], dt.int16, name=f"sdi{k}")
                nc.vector.tensor_copy(
                    out=sdi[:, :].bitcast(dt.bfloat16),
                    in_=ibx[:, 0 : nd // 16],
                )
                sdis.append(sdi)

            for k, (b, lo, cols, sbase, ibase, calls) in enumerate(plan):
                ncols128 = cols // 128
                sb_, sl_ = _window(b >> 2)
                db_, dl_ = _window(b & 3)
                sdi = sdis[k]
                xs = xpool.tile([128, CMAX], dt.bfloat16, tag="xs")
                xd = xpool.tile([128, CMAX], dt.bfloat16, tag="xd")
                for side, paired, coff, cn, ioff in calls:
                    tile_, wb, wl = (
                        (xs, sb_, sl_) if side == "s" else (xd, db_, dl_)
                    )
                    if paired:
                        n = cn // 2
                        pi = sdi[:, ioff // 16 : (ioff + n) // 16]
                        base = emd[wb : wb + wl, :]
                        pview = AP(
                            base.tensor, base.offset,
                            [[128, wl - 1], [1, 256]],
                        )
                        t = tile_[:, coff : coff + cn]
                        out3 = AP(
                            t.tensor, t.offset,
                            [[t.ap[0][0], 128], [n, 2], [1, n]],
                        )
                        nc.gpsimd.dma_gather(
                            out3, pview, pi, n, n, 256,
                            elem_step=128, transpose=True, single_packet=False,
                        )
                    else:
                        si = sdi[:, ioff // 16 : (ioff + cn) // 16]
                        nc.gpsimd.dma_gather(
                            tile_[:, coff : coff + cn].unsqueeze(1),
                            emd[wb : wb + wl, :], si, cn, cn, D,
                            transpose=True, single_packet=False,
                        )

                lg = plp.tile([128, CMAX // 128], dt.float32, tag="lg")
                ntile = (cols + 511) // 512
                # software pipeline: L2 of tile t issues L2LAG tiles late
                hq = []
                for t in range(ntile + L2LAG):
                    if t < ntile:
                        e0 = t * 512
                        n = min(512, cols - e0)
                        h0p = php.tile([128, 512], dt.float32, tag="h0p")
                        h1p = php.tile([128, 512], dt.float32, tag="h1p")
                        nc.tensor.matmul(
                            h0p[:, 0:n], lhsT=w1_sb[:, 0:128],
                            rhs=xs[:, e0 : e0 + n], start=True, stop=False,
                        )
                        nc.tensor.matmul(
                            h0p[:, 0:n], lhsT=w1_sb[:, 256:384],
                            rhs=xd[:, e0 : e0 + n], start=False, stop=True,
                        )
                        nc.tensor.matmul(
                            h1p[:, 0:n], lhsT=w1_sb[:, 128:256],
                            rhs=xs[:, e0 : e0 + n], start=True, stop=False,
                        )
                        nc.tensor.matmul(
                            h1p[:, 0:n], lhsT=w1_sb[:, 384:512],
                            rhs=xd[:, e0 : e0 + n], start=False, stop=True,
                        )
                        h0s = hpool.tile([128, 512], dt.bfloat16, tag="h0s")
                        h1s = hpool.tile([128, 512], dt.bfloat16, tag="h1s")
                        nc.scalar.activation(
                            h0s[:, 0:n], h0p[:, 0:n], AF.Relu, bias=b1_sb[:, 0:1]
                        )
                        nc.vector.tensor_scalar(
                            h1s[:, 0:n], h1p[:, 0:n],
                            b1_sb[:, 1:2], 0.0, ALU.add, ALU.max,
                        )
                        hq.append((t, n, h0s, h1s))
                    if t >= L2LAG:
                        pt, pn, p0, p1 = hq[t - L2LAG]
                        for s in range((pn + 127) // 128):
                            ns = min(128, pn - s * 128)
                            col = pt * 4 + s
                            nc.tensor.matmul(
                                lg[0:ns, col : col + 1],
                                lhsT=p0[:, s * 128 : s * 128 + ns],
                                rhs=w2_sb[:, 0:1], start=True, stop=False,
                            )
                            nc.tensor.matmul(
                                lg[0:ns, col : col + 1],
                                lhsT=p1[:, s * 128 : s * 128 + ns],
                                rhs=w2_sb[:, 1:2], start=False, stop=True,
                            )
                nc.scalar.activation(
                    osb[:, sbase // 128 : sbase // 128 + ncols128],
                    lg[:, 0:ncols128], AF.Sigmoid, bias=b2_sb[:, 0:1],
                )
                if k == len(plan) * MIDSPLIT_N // MIDSPLIT_D and k < len(plan) - 2:
                    midcol = (sbase + cols) // 128
                    nc.sync.dma_start(out_d[:, 0:midcol], osb[:, 0:midcol])
                if k == len(plan) - 2 and midcol < (sbase + cols) // 128:
                    nextcol = (sbase + cols) // 128
                    nc.sync.dma_start(
                        out_d[:, midcol:nextcol], osb[:, midcol:nextcol]
                    )
                    midcol = nextcol
                if k == len(plan) - 1 and midcol < TOT:
                    nc.sync.dma_start(
                        out_d[:, midcol:TOT], osb[:, midcol:TOT]
                    )

    nc.compile()
    _NC_CACHE[caps] = nc
    return nc


def _wrap_idx(vals):
    """int16 [n] -> [16, n//16] wrapped in 16 partitions."""
    n = vals.shape[0]
    return np.ascontiguousarray(vals.reshape(n // 16, 16).T)


def _path_greedy(pos_vals):
    """Pair refs at adjacent window-local positions (p, p+1), each ref used
    once, left-to-right greedy with carry (optimal for paths).

    pos_vals: int array of window-local positions. Returns (a_idx, b_idx):
    indices into pos_vals; ref a at position p pairs with ref b at p+1.
    """
    order = np.argsort(pos_vals, kind="stable")
    pv = pos_vals[order]
    cnt = np.bincount(pv, minlength=WIN)
    starts = np.zeros(WIN + 1, np.int64)
    np.cumsum(cnt, out=starts[1:])
    pairs_a = []
    pairs_b = []
    avail = 0
    prev_p = -2
    for p in np.nonzero(cnt)[0]:
        c = int(cnt[p])
        s = int(starts[p])
        if p == prev_p + 1 and avail > 0:
            t = min(avail, c)
            ps = int(starts[prev_p])
            pc = int(cnt[prev_p])
            pairs_a.append(order[ps + pc - avail : ps + pc - avail + t])
            pairs_b.append(order[s : s + t])
            avail = c - t
        else:
            avail = c
        prev_p = p
    if pairs_a:
        return np.concatenate(pairs_a), np.concatenate(pairs_b)
    return np.empty(0, np.int64), np.empty(0, np.int64)


def _global_plan(ei):
    """Bucket + pair all edges globally, deal to cores.

    Returns (caps, per_core) where caps = (sp[16], dp[16], so[16]) (sp/dp in
    pairs, so in cols) and per_core[c][b] = (sp_pairs [p,2], dp_pairs [p,2],
    solo [s]) holding GLOBAL edge ids."""
    src = ei[:, 0].astype(np.int64)
    dst = ei[:, 1].astype(np.int64)
    bucket = (src // WIN) * 4 + dst // WIN

    spc = [0] * NBUCK
    dpc = [0] * NBUCK
    soc = [0] * NBUCK
    per_core = [dict() for _ in range(NCORES)]
    for b in range(NBUCK):
        sel = np.nonzero(bucket == b)[0]
        a, bb = _path_greedy(src[sel] % WIN)
        paired = np.zeros(len(sel), bool)
        paired[a] = True
        paired[bb] = True
        sp = np.stack([sel[a], sel[bb]], axis=1) if len(a) else \
            np.empty((0, 2), np.int64)
        left = np.nonzero(~paired)[0]
        a2, b2 = _path_greedy(dst[sel[left]] % WIN)
        lp = np.zeros(len(left), bool)
        lp[a2] = True
        lp[b2] = True
        dp = np.stack([sel[left[a2]], sel[left[b2]]], axis=1) if len(a2) else \
            np.empty((0, 2), np.int64)
        so = sel[left[~lp]]

        # pair caps: keep exactly NCORES*cap pairs (cap 128-aligned), demote
        # the rest globally into the solo pool; deal pairs and solos round-
        # robin so every core gets exactly cap pairs and solos within +-1.
        # Solo cap rounds up to 128 cols (256 for the split first/last
        # buckets so region boundaries stay 256-aligned under col cuts).
        nsp = len(sp) // NCORES // 128 * 128
        ndp = len(dp) // NCORES // 128 * 128
        spc[b] = nsp
        dpc[b] = ndp
        keep_sp = sp[: nsp * NCORES]
        keep_dp = dp[: ndp * NCORES]
        solo_pool = np.concatenate(
            [sp[nsp * NCORES :].reshape(-1), dp[ndp * NCORES :].reshape(-1),
             so]
        )
        max_solo = 0
        for c in range(NCORES):
            per_core[c][b] = (
                keep_sp[c::NCORES], keep_dp[c::NCORES], solo_pool[c::NCORES]
            )
            max_solo = max(max_solo, len(solo_pool[c::NCORES]))
        gran = 256 if b in (0, NBUCK - 1) else 128
        soc[b] = -(-max_solo // gran) * gran
    return (tuple(spc), tuple(dpc), tuple(soc)), per_core


def _prepare_core(core_items, caps, plan, EP, DTOT, src, dst):
    """Build one core's wrapped idx array + slot->edge map for the shared
    chunk plan. core_items: dict b -> (sp, dp, solo) global-edge-id arrays."""
    spc, dpc, soc = caps
    idx = np.zeros((16, DTOT // 16), np.int16)
    edge_of_slot = np.full(EP, -1, np.int64)

    # per-bucket col -> edge map (built once per bucket)
    col_edge = {}
    for b in range(NBUCK):
        sp, dp, solo = core_items[b]
        cols = _bucket_cols(caps, b)
        ce = np.full(cols, -1, np.int64)
        r1 = 2 * spc[b]
        r2 = r1 + soc[b]
        # SP region pair p -> cols (2-aligned within each CALL, but the call
        # layout depends on the chunk split; store pairs positionally and let
        # the per-call loop place them)
        col_edge[b] = ce  # filled below per chunk for pair regions
        ce[r1 : r1 + len(solo)] = solo

    for b, lo, cols, sbase, ibase, calls in plan:
        sp, dp, solo = core_items[b]
        sb_ = (b >> 2) * WIN
        db_ = (b & 3) * WIN
        r1 = 2 * spc[b]
        r2 = r1 + soc[b]
        ce = col_edge[b]
        # place pair edges for pair calls of this chunk (sub-block layout is
        # per-call: first rows at cols [coff, coff+n), second rows at
        # [coff+n, coff+2n))
        for side, paired, coff, cn, ioff in calls:
            if not paired:
                continue
            n = cn // 2
            glo = lo + coff
            pairs = sp if side == "s" else dp
            p0 = (glo - (0 if side == "s" else r2)) // 2
            pc_ = pairs[p0 : p0 + n]
            m = len(pc_)
            if m:
                ce[glo : glo + m] = pc_[:, 0]
                ce[glo + n : glo + n + m] = pc_[:, 1]
        # now write idx values for every call
        for side, paired, coff, cn, ioff in calls:
            tab = src if side == "s" else dst
            base_ = sb_ if side == "s" else db_
            if paired:
                n = cn // 2
                e = ce[lo + coff : lo + coff + n]
                v = np.zeros(n, np.int16)
                m = e >= 0
                v[m] = (tab[e[m]] - base_).astype(np.int16)
                idx[:, (ibase + ioff) // 16 : (ibase + ioff + n) // 16] = \
                    _wrap_idx(v)
            else:
                e = ce[lo + coff : lo + coff + cn]
                v = np.zeros(cn, np.int16)
                m = e >= 0
                v[m] = (tab[e[m]] - base_).astype(np.int16)
                idx[:, (ibase + ioff) // 16 : (ibase + ioff + cn) // 16] = \
                    _wrap_idx(v)
        edge_of_slot[sbase : sbase + cols] = ce[lo : lo + cols]
    return idx, edge_of_slot


def kernel(emd_all, edge_index, W1, b1, W2, b2):
    global LAST_RESULTS
    emd_bf = np.ascontiguousarray(np.asarray(emd_all, dtype=np.float32)).astype(BF16)
    ei = np.asarray(edge_index).astype(np.int64)
    W1 = np.asarray(W1, dtype=np.float32)
    W2 = np.asarray(W2, dtype=np.float32)
    b1 = np.asarray(b1, dtype=np.float32).reshape(-1)
    b2 = np.asarray(b2, dtype=np.float32).reshape(-1)
    src = ei[:, 0].astype(np.int64)
    dst = ei[:, 1].astype(np.int64)

    caps, per_core = _global_plan(ei)
    plan, EP, DTOT = _plan_sizes(caps)

    # lhsT blocks: [src->h0, src->h1, dst->h0, dst->h1]
    w1_arr = np.concatenate(
        [W1[:D, :D], W1[:D, D:], W1[D:, :D], W1[D:, D:]], axis=1
    ).astype(BF16)
    b1_arr = np.ascontiguousarray(np.stack([b1[:128], b1[128:]], axis=1))
    w2_arr = np.ascontiguousarray(np.stack([W2[:128, 0], W2[128:, 0]], axis=1)).astype(
        BF16
    )
    b2_arr = np.full((128, 1), b2[0], np.float32)
    sel_arr = np.zeros((16, 128), np.float32)
    sel_arr[np.arange(128) % 16, np.arange(128)] = 1.0
    sel_arr = sel_arr.astype(BF16).view(np.int16)

    in_maps = []
    unshard = []
    for c in range(NCORES):
        idx, edge_of_slot = _prepare_core(
            per_core[c], caps, plan, EP, DTOT, src, dst
        )
        unshard.append(edge_of_slot)
        in_maps.append(
            {
                "emd": emd_bf,
                "idx": np.concatenate([sel_arr, idx], axis=1),
                "w1": w1_arr,
                "b1": b1_arr,
                "w2": w2_arr,
                "b2": b2_arr,
            }
        )

    nc = _build_program(caps)
    res = run_bass_kernel_spmd(nc, in_maps, core_ids=list(range(NCORES)))
    LAST_RESULTS = res

    y = np.empty((E_TOTAL,), np.float32)
    for c in range(NCORES):
        edge_of_slot = unshard[c]  # slot -> global edge id
        out = np.asarray(res.results[c]["out"], dtype=np.float32)  # [128, TOT]
        flat = out.T.reshape(-1)  # slot-ordered
        mask = edge_of_slot >= 0
        y[edge_of_slot[mask]] = flat[mask]
    return y.reshape(E_TOTAL, 1)


if __name__ == "__main__":
    rng = np.random.default_rng(0)
    emd = rng.standard_normal((N_NODES, D), dtype=np.float32)
    ei = rng.integers(0, N_NODES, size=(E_TOTAL, 2)).astype(np.int32)
    W1 = rng.standard_normal((2 * D, H), dtype=np.float32) / np.sqrt(2 * D)
    W2 = rng.standard_normal((H, 1), dtype=np.float32) / np.sqrt(H)
    out = kernel(emd, ei, W1, np.zeros(H, np.float32), W2, np.zeros(1, np.float32))
    print(out.shape, out[:4, 0])


# revision 18
# speedup vs baseline: 1.2126x; 1.0198x over previous
"""Link-predictor GNN kernel for 8 TRN2 NeuronCores.

Strategy (per sharding hint): shard edges across 8 cores (data parallel),
replicate the bf16 node-embedding table + MLP weights on every core.

Edges are bucketed by (src_window, dst_window) where a window is 25000
table rows (4 windows cover 100000 nodes) so window-relative node ids fit
the int16 indices of the batched SWDGE dma_gather.

DMA-descriptor reduction via 512B paired descriptors: the SWDGE cost is
per-descriptor with a 2x penalty under 512 bytes, so a 256-element bf16
descriptor (elem_step=128: table rows u and u+1) costs the same as a
128-element one but carries two rows. Per bucket, edges whose SRC rows are
adjacent (u, u+1) are paired globally (path-greedy along each window),
leftovers are paired again by DST adjacency, and the rest stay solo:
~85% of edges land in pairs, cutting gather descriptors per edge from
2.0 to ~1.6. A paired call of n pairs lands [128, 2, n] (sub-block 0 =
first rows, sub-block 1 = second rows, both in X^T layout); the other
side uses a per-slot elem=128 gather. Pairs/solos are dealt round-robin
to cores so the shared static per-bucket capacities are tight.

Each bucket's [SP pairs | solo | DP pairs] column layout is processed as
ONE chunk with 4 merged gather calls (src-pair, src-solo over solo+DP,
dst-solo over SP+solo, dst-pair), keeping the Pool engine's ~1us fixed
SWDGE overhead per call well below the DMA transfer time so descriptor
generation always runs ahead. The first and last buckets are split into
small pieces so the DMA pipeline fills fast and drains short.

MLP per 512-edge tile: h = relu(W1s^T Xs + W1d^T Xd + b1) via 4 matmuls
accumulating in PSUM; relu of h-half-0 on ACT (bias fused), half-1 on DVE
(tensor_scalar add+max). Layer 2 contracts h against W2 using h-subtiles
as the stationary operand: 2 matmuls of N=1 per 128-edge subtile writing
one PSUM column; a whole chunk's logits accumulate into one PSUM tile so a
single sigmoid + one small DMA per chunk emits [128, cols] f32 results.
Host inverts the slot permutation. The 16-partition-wrapped index arrays
are loaded once and replicated to the 128 partitions the gather hardware
expects via 0/1-matmul broadcasts of the raw bf16 bit patterns.
"""

import sys

sys.path.insert(0, "/opt/trn_rl_repo")

import numpy as np
import ml_dtypes

from concourse import bacc, mybir, tile
from concourse.ap import AP
from concourse.bass_utils import run_bass_kernel_spmd

BF16 = ml_dtypes.bfloat16

N_NODES = 100000
D = 128
H = 256
E_TOTAL = 600000
NCORES = 8
WIN = 25000                      # table-row window (< 2^15 for int16 idx)
NBUCK = 16                       # 4 src windows x 4 dst windows
XBUFS = 6                        # gather buffer depth per side
HPBUFS = 2                       # PSUM h depth
L2LAG = 1                        # tiles of lag between L1 and L2 issue
HBUFS = 3                        # h sbuf tile depth
PLBUFS = 2                       # logits PSUM depth
PBBUFS = 2                       # idx-broadcast PSUM depth
OBUFS = 3                        # sigmoid output tile depth
FILL_PIECES = (256,)             # leading col-cuts of the first bucket
TAIL_PIECE = 1024                # trailing bucket split granularity
MIDSPLITS = 2                    # extra column cuts per middle bucket
MIDSPLIT_N, MIDSPLIT_D = 1, 4    # mid output-store point (fraction of plan)

# (sp_pairs, dp_pairs, solo_cols) per bucket for the canonical
# setup_inputs() edge set. kernel() recomputes these from its actual
# inputs; this default only serves _build_program() callers that have no
# inputs (e.g. a standalone TimelineSim of the program).
DEFAULT_CAPS = (
    (1664, 1664, 1664, 1664, 1664, 1664, 1664, 1664,
     1664, 1664, 1664, 1664, 1664, 1664, 1664, 1664),
    (256, 256, 256, 256, 256, 256, 256, 256,
     256, 256, 256, 256, 256, 256, 256, 256),
    (1024, 896, 896, 1024, 896, 896, 896, 896,
     896, 896, 896, 896, 896, 896, 896, 1024),
)  # matches _global_plan(setup_inputs()['edge_index'])

LAST_RESULTS = None
_NC_CACHE: dict = {}


def _window(w):
    base = w * WIN
    return base, min(WIN, N_NODES - base)


def _bucket_cols(caps, b):
    spc, dpc, soc = caps
    return 2 * spc[b] + soc[b] + 2 * dpc[b]


def _calls_for_range(caps, b, lo, hi):
    """Gather calls covering chunk-relative cols [lo, hi) of bucket b's
    [SP | solo | DP] layout. Returns [(side, paired, col_off, ncols)] with
    col_off relative to lo; adjacent same-(side,paired) solo ranges merged.
    Cuts must be 256-aligned so pair calls keep num_idxs % 128 == 0."""
    spc, dpc, soc = caps
    r1 = 2 * spc[b]
    r2 = r1 + soc[b]
    r3 = r2 + 2 * dpc[b]
    assert 0 <= lo < hi <= r3 and lo % 256 == 0 and (hi % 256 == 0 or hi == r3)
    # call order matters for pipelining: [s-pair, d-solo] first makes the SP
    # region (the bulk of the chunk) compute-ready before the solo/DP calls
    # finish, so the MLP tracks the DMA stream with a short lag.
    calls = []
    a, bnd = max(lo, 0), min(hi, r1)
    if a < bnd:
        calls.append(("s", True, a - lo, bnd - a))
    a, bnd = max(lo, 0), min(hi, r2)
    if a < bnd:
        calls.append(("d", False, a - lo, bnd - a))
    a, bnd = max(lo, r1), min(hi, r3)
    if a < bnd:
        calls.append(("s", False, a - lo, bnd - a))
    a, bnd = max(lo, r2), min(hi, r3)
    if a < bnd:
        calls.append(("d", True, a - lo, bnd - a))
    return calls


def _chunk_plan(caps):
    """Ordered chunk list. Chunk = (bucket, lo, cols, slot_base, idx_base,
    calls) where calls = [(side, paired, col_off, ncols, idx_off)], idx_off
    in descriptors relative to idx_base."""
    pieces = []  # (bucket, lo, hi)
    for b in range(NBUCK):
        cols = _bucket_cols(caps, b)
        if b == 0:
            o = 0
            for fp in FILL_PIECES:
                if o + fp < cols:
                    pieces.append((b, o, o + fp))
                    o += fp
            if o < cols:
                pieces.append((b, o, cols))
        elif b == NBUCK - 1:
            # pieces from the end: ..., 1024, 512, 256 so the drain is short
            cuts = [cols]
            rem = cols
            for tp in (256, 512):
                if rem - tp > 0:
                    rem -= tp
                    cuts.append(rem)
            while rem - TAIL_PIECE > 0:
                rem -= TAIL_PIECE
                cuts.append(rem)
            cuts.append(0)
            cuts = sorted(set(cuts))
            for i in range(len(cuts) - 1):
                pieces.append((b, cuts[i], cuts[i + 1]))
        else:
            cuts = sorted(
                {0, cols}
                | {cols * i // (MIDSPLITS + 1) // 256 * 256
                   for i in range(1, MIDSPLITS + 1)}
            )
            for i in range(len(cuts) - 1):
                pieces.append((b, cuts[i], cuts[i + 1]))
    plan = []
    sbase = 0
    ibase = 0
    for b, lo, hi in pieces:
        calls = []
        ioff = 0
        for side, paired, coff, ncols in _calls_for_range(caps, b, lo, hi):
            nd = ncols // 2 if paired else ncols
            calls.append((side, paired, coff, ncols, ioff))
            ioff += nd
        plan.append((b, lo, hi - lo, sbase, ibase, calls))
        sbase += hi - lo
        ibase += ioff
    return plan


def _plan_sizes(caps):
    plan = _chunk_plan(caps)
    ep = sum(p[2] for p in plan)
    last = plan[-1]
    dtot = last[4] + sum(
        (nc // 2 if paired else nc) for _, paired, _, nc, _ in last[5]
    )
    return plan, ep, dtot


def _build_program(caps=None):
    if caps is None:
        caps = DEFAULT_CAPS
    caps = tuple(tuple(int(x) for x in c) for c in caps)
    if caps in _NC_CACHE:
        return _NC_CACHE[caps]

    dt = mybir.dt
    AF = mybir.ActivationFunctionType
    ALU = mybir.AluOpType

    plan, EP, DTOT = _plan_sizes(caps)
    TOT = EP // 128
    CMAX = max(p[2] for p in plan)
    NDMAX = max(
        sum(nc // 2 if paired else nc for _, paired, _, nc, _ in p[5])
        for p in plan
    )

    nc = bacc.Bacc(
        "TRN2",
        target_bir_lowering=False,
        debug=False,
        enable_asserts=False,
        num_devices=NCORES,
    )
    emd = nc.dram_tensor("emd", [N_NODES, D], dt.bfloat16, kind="ExternalInput")
    idx_d = nc.dram_tensor(
        "idx", [16, 128 + DTOT // 16], dt.int16, kind="ExternalInput"
    )
    w1_d = nc.dram_tensor("w1", [128, 512], dt.bfloat16, kind="ExternalInput")
    b1_d = nc.dram_tensor("b1", [128, 2], dt.float32, kind="ExternalInput")
    w2_d = nc.dram_tensor("w2", [128, 2], dt.bfloat16, kind="ExternalInput")
    b2_d = nc.dram_tensor("b2", [128, 1], dt.float32, kind="ExternalInput")
    out_d = nc.dram_tensor("out", [128, TOT], dt.float32, kind="ExternalOutput")

    with tile.TileContext(nc) as tc:
        with (
            tc.tile_pool(name="const", bufs=1) as cpool,
            tc.tile_pool(name="x", bufs=XBUFS) as xpool,
            tc.tile_pool(name="h", bufs=HBUFS) as hpool,
            tc.tile_pool(name="o", bufs=OBUFS) as opool,
            tc.tile_pool(name="ph", bufs=HPBUFS, space="PSUM") as php,
            tc.tile_pool(name="pl", bufs=PLBUFS, space="PSUM") as plp,
            tc.tile_pool(name="pb", bufs=PBBUFS, space="PSUM") as pbp,
        ):
            # one combined load: the 0/1 selection matrix (bf16-bitcast) in
            # cols 0:128, then all (16-partition-wrapped) gather indices.
            selidx_sb = cpool.tile([16, 128 + DTOT // 16], dt.int16)
            nc.sync.dma_start(selidx_sb[:, :], idx_d[:, :])
            sel_sb = selidx_sb[:, 0:128].bitcast(dt.bfloat16)
            idx16_sb = selidx_sb[:, 128:]
            osb = cpool.tile([128, TOT], dt.float32)
            w1_sb = cpool.tile([128, 512], dt.bfloat16)
            nc.scalar.dma_start(w1_sb[:, :], w1_d[:, :])
            b1_sb = cpool.tile([128, 2], dt.float32)
            nc.scalar.dma_start(b1_sb[:, :], b1_d[:, :])
            w2_sb = cpool.tile([128, 2], dt.bfloat16)
            nc.scalar.dma_start(w2_sb[:, :], w2_d[:, :])
            b2_sb = cpool.tile([128, 1], dt.float32)
            nc.scalar.dma_start(b2_sb[:, :], b2_d[:, :])

            # broadcast every chunk's indices to 128 partitions up front;
            # PE/DVE pipeline stays far ahead of the gathers consuming them
            midcol = 0
            sdis = []
            for k, (b, lo, cols, sbase, ibase, calls) in enumerate(plan):
                nd = sum(c // 2 if p else c for _, p, _, c, _ in calls)
                c16 = ibase // 16
                ibx = pbp.tile([128, NDMAX // 16], dt.float32, tag="ibx")
                nc.tensor.matmul(
                    ibx[:, 0 : nd // 16], lhsT=sel_sb,
                    rhs=idx16_sb[:, c16 : c16 + nd // 16].bitcast(dt.bfloat16),
                    start=True, stop=True,
                )
                sdi = cpool.tile([128, nd // 16], dt.int16, name=f"sdi{k}")
                nc.vector.tensor_copy(
                    out=sdi[:, :].bitcast(dt.bfloat16),
                    in_=ibx[:, 0 : nd // 16],
                )
                sdis.append(sdi)

            for k, (b, lo, cols, sbase, ibase, calls) in enumerate(plan):
                ncols128 = cols // 128
                sb_, sl_ = _window(b >> 2)
                db_, dl_ = _window(b & 3)
                sdi = sdis[k]
                xs = xpool.tile([128, CMAX], dt.bfloat16, tag="xs")
                xd = xpool.tile([128, CMAX], dt.bfloat16, tag="xd")
                for side, paired, coff, cn, ioff in calls:
                    tile_, wb, wl = (
                        (xs, sb_, sl_) if side == "s" else (xd, db_, dl_)
                    )
                    if paired:
                        n = cn // 2
                        pi = sdi[:, ioff // 16 : (ioff + n) // 16]
                        base = emd[wb : wb + wl, :]
                        pview = AP(
                            base.tensor, base.offset,
                            [[128, wl - 1], [1, 256]],
                        )
                        t = tile_[:, coff : coff + cn]
                        out3 = AP(
                            t.tensor, t.offset,
                            [[t.ap[0][0], 128], [n, 2], [1, n]],
                        )
                        nc.gpsimd.dma_gather(
                            out3, pview, pi, n, n, 256,
                            elem_step=128, transpose=True, single_packet=False,
                        )
                    else:
                        si = sdi[:, ioff // 16 : (ioff + cn) // 16]
                        nc.gpsimd.dma_gather(
                            tile_[:, coff : coff + cn].unsqueeze(1),
                            emd[wb : wb + wl, :], si, cn, cn, D,
                            transpose=True, single_packet=False,
                        )

                lg = plp.tile([128, CMAX // 128], dt.float32, tag="lg")
                ntile = (cols + 511) // 512
                # software pipeline: L2 of tile t issues L2LAG tiles late
                hq = []
                for t in range(ntile + L2LAG):
                    if t < ntile:
                        e0 = t * 512
                        n = min(512, cols - e0)
                        h0p = php.tile([128, 512], dt.float32, tag="h0p")
                        h1p = php.tile([128, 512], dt.float32, tag="h1p")
                        nc.tensor.matmul(
                            h0p[:, 0:n], lhsT=w1_sb[:, 0:128],
                            rhs=xs[:, e0 : e0 + n], start=True, stop=False,
                        )
                        nc.tensor.matmul(
                            h0p[:, 0:n], lhsT=w1_sb[:, 256:384],
                            rhs=xd[:, e0 : e0 + n], start=False, stop=True,
                        )
                        nc.tensor.matmul(
                            h1p[:, 0:n], lhsT=w1_sb[:, 128:256],
                            rhs=xs[:, e0 : e0 + n], start=True, stop=False,
                        )
                        nc.tensor.matmul(
                            h1p[:, 0:n], lhsT=w1_sb[:, 384:512],
                            rhs=xd[:, e0 : e0 + n], start=False, stop=True,
                        )
                        h0s = hpool.tile([128, 512], dt.bfloat16, tag="h0s")
                        h1s = hpool.tile([128, 512], dt.bfloat16, tag="h1s")
                        nc.scalar.activation(
                            h0s[:, 0:n], h0p[:, 0:n], AF.Relu, bias=b1_sb[:, 0:1]
                        )
                        nc.vector.tensor_scalar(
                            h1s[:, 0:n], h1p[:, 0:n],
                            b1_sb[:, 1:2], 0.0, ALU.add, ALU.max,
                        )
                        hq.append((t, n, h0s, h1s))
                    if t >= L2LAG:
                        pt, pn, p0, p1 = hq[t - L2LAG]
                        for s in range((pn + 127) // 128):
                            ns = min(128, pn - s * 128)
                            col = pt * 4 + s
                            nc.tensor.matmul(
                                lg[0:ns, col : col + 1],
                                lhsT=p0[:, s * 128 : s * 128 + ns],
                                rhs=w2_sb[:, 0:1], start=True, stop=False,
                            )
                            nc.tensor.matmul(
                                lg[0:ns, col : col + 1],
                                lhsT=p1[:, s * 128 : s * 128 + ns],
                                rhs=w2_sb[:, 1:2], start=False, stop=True,
                            )
                nc.scalar.activation(
                    osb[:, sbase // 128 : sbase // 128 + ncols128],
                    lg[:, 0:ncols128], AF.Sigmoid, bias=b2_sb[:, 0:1],
                )
                if k == len(plan) * MIDSPLIT_N // MIDSPLIT_D and k < len(plan) - 2:
                    midcol = (sbase + cols) // 128
                    nc.sync.dma_start(out_d[:, 0:midcol], osb[:, 0:midcol])
                if k == len(plan) - 2 and midcol < (sbase + cols) // 128:
                    nextcol = (sbase + cols) // 128
                    nc.sync.dma_start(
                        out_d[:, midcol:nextcol], osb[:, midcol:nextcol]
                    )
                    midcol = nextcol
                if k == len(plan) - 1 and midcol < TOT:
                    nc.sync.dma_start(
                        out_d[:, midcol:TOT], osb[:, midcol:TOT]
                    )

    nc.compile()
    _NC_CACHE[caps] = nc
    return nc


def _wrap_idx(vals):
    """int16 [n] -> [16, n//16] wrapped in 16 partitions."""
    n = vals.shape[0]
    return np.ascontiguousarray(vals.reshape(n // 16, 16).T)


def _path_greedy(pos_vals):
    """Pair refs at adjacent window-local positions (p, p+1), each ref used
    once, left-to-right greedy with carry (optimal for paths).

    pos_vals: int array of window-local positions. Returns (a_idx, b_idx):
    indices into pos_vals; ref a at position p pairs with ref b at p+1.
    """
    order = np.argsort(pos_vals, kind="stable")
    pv = pos_vals[order]
    cnt = np.bincount(pv, minlength=WIN)
    starts = np.zeros(WIN + 1, np.int64)
    np.cumsum(cnt, out=starts[1:])
    pairs_a = []
    pairs_b = []
    avail = 0
    prev_p = -2
    for p in np.nonzero(cnt)[0]:
        c = int(cnt[p])
        s = int(starts[p])
        if p == prev_p + 1 and avail > 0:
            t = min(avail, c)
            ps = int(starts[prev_p])
            pc = int(cnt[prev_p])
            pairs_a.append(order[ps + pc - avail : ps + pc - avail + t])
            pairs_b.append(order[s : s + t])
            avail = c - t
        else:
            avail = c
        prev_p = p
    if pairs_a:
        return np.concatenate(pairs_a), np.concatenate(pairs_b)
    return np.empty(0, np.int64), np.empty(0, np.int64)


def _global_plan(ei):
    """Bucket + pair all edges globally, deal to cores.

    Returns (caps, per_core) where caps = (sp[16], dp[16], so[16]) (sp/dp in
    pairs, so in cols) and per_core[c][b] = (sp_pairs [p,2], dp_pairs [p,2],
    solo [s]) holding GLOBAL edge ids."""
    src = ei[:, 0].astype(np.int64)
    dst = ei[:, 1].astype(np.int64)
    bucket = (src // WIN) * 4 + dst // WIN

    spc = [0] * NBUCK
    dpc = [0] * NBUCK
    soc = [0] * NBUCK
    per_core = [dict() for _ in range(NCORES)]
    for b in range(NBUCK):
        sel = np.nonzero(bucket == b)[0]
        a, bb = _path_greedy(src[sel] % WIN)
        paired = np.zeros(len(sel), bool)
        paired[a] = True
        paired[bb] = True
        sp = np.stack([sel[a], sel[bb]], axis=1) if len(a) else \
            np.empty((0, 2), np.int64)
        left = np.nonzero(~paired)[0]
        a2, b2 = _path_greedy(dst[sel[left]] % WIN)
        lp = np.zeros(len(left), bool)
        lp[a2] = True
        lp[b2] = True
        dp = np.stack([sel[left[a2]], sel[left[b2]]], axis=1) if len(a2) else \
            np.empty((0, 2), np.int64)
        so = sel[left[~lp]]

        # pair caps: keep exactly NCORES*cap pairs (cap 128-aligned), demote
        # the rest globally into the solo pool; deal pairs and solos round-
        # robin so every core gets exactly cap pairs and solos within +-1.
        # Solo cap rounds up to 128 cols (256 for the split first/last
        # buckets so region boundaries stay 256-aligned under col cuts).
        nsp = len(sp) // NCORES // 128 * 128
        ndp = len(dp) // NCORES // 128 * 128
        spc[b] = nsp
        dpc[b] = ndp
        keep_sp = sp[: nsp * NCORES]
        keep_dp = dp[: ndp * NCORES]
        solo_pool = np.concatenate(
            [sp[nsp * NCORES :].reshape(-1), dp[ndp * NCORES :].reshape(-1),
             so]
        )
        max_solo = 0
        for c in range(NCORES):
            per_core[c][b] = (
                keep_sp[c::NCORES], keep_dp[c::NCORES], solo_pool[c::NCORES]
            )
            max_solo = max(max_solo, len(solo_pool[c::NCORES]))
        gran = 256 if b in (0, NBUCK - 1) else 128
        soc[b] = -(-max_solo // gran) * gran
    return (tuple(spc), tuple(dpc), tuple(soc)), per_core


def _prepare_core(core_items, caps, plan, EP, DTOT, src, dst):
    """Build one core's wrapped idx array + slot->edge map for the shared
    chunk plan. core_items: dict b -> (sp, dp, solo) global-edge-id arrays."""
    spc, dpc, soc = caps
    idx = np.zeros((16, DTOT // 16), np.int16)
    edge_of_slot = np.full(EP, -1, np.int64)

    # per-bucket col -> edge map (built once per bucket)
    col_edge = {}
    for b in range(NBUCK):
        sp, dp, solo = core_items[b]
        cols = _bucket_cols(caps, b)
        ce = np.full(cols, -1, np.int64)
        r1 = 2 * spc[b]
        r2 = r1 + soc[b]
        # SP region pair p -> cols (2-aligned within each CALL, but the call
        # layout depends on the chunk split; store pairs positionally and let
        # the per-call loop place them)
        col_edge[b] = ce  # filled below per chunk for pair regions
        ce[r1 : r1 + len(solo)] = solo

    for b, lo, cols, sbase, ibase, calls in plan:
        sp, dp, solo = core_items[b]
        sb_ = (b >> 2) * WIN
        db_ = (b & 3) * WIN
        r1 = 2 * spc[b]
        r2 = r1 + soc[b]
        ce = col_edge[b]
        # place pair edges for pair calls of this chunk (sub-block layout is
        # per-call: first rows at cols [coff, coff+n), second rows at
        # [coff+n, coff+2n))
        for side, paired, coff, cn, ioff in calls:
            if not paired:
                continue
            n = cn // 2
            glo = lo + coff
            pairs = sp if side == "s" else dp
            p0 = (glo - (0 if side == "s" else r2)) // 2
            pc_ = pairs[p0 : p0 + n]
            m = len(pc_)
            if m:
                ce[glo : glo + m] = pc_[:, 0]
                ce[glo + n : glo + n + m] = pc_[:, 1]
        # now write idx values for every call
        for side, paired, coff, cn, ioff in calls:
            tab = src if side == "s" else dst
            base_ = sb_ if side == "s" else db_
            if paired:
                n = cn // 2
                e = ce[lo + coff : lo + coff + n]
                v = np.zeros(n, np.int16)
                m = e >= 0
                v[m] = (tab[e[m]] - base_).astype(np.int16)
                idx[:, (ibase + ioff) // 16 : (ibase + ioff + n) // 16] = \
                    _wrap_idx(v)
            else:
                e = ce[lo + coff : lo + coff + cn]
                v = np.zeros(cn, np.int16)
                m = e >= 0
                v[m] = (tab[e[m]] - base_).astype(np.int16)
                idx[:, (ibase + ioff) // 16 : (ibase + ioff + cn) // 16] = \
                    _wrap_idx(v)
        edge_of_slot[sbase : sbase + cols] = ce[lo : lo + cols]
    return idx, edge_of_slot


def kernel(emd_all, edge_index, W1, b1, W2, b2):
    global LAST_RESULTS
    emd_bf = np.ascontiguousarray(np.asarray(emd_all, dtype=np.float32)).astype(BF16)
    ei = np.asarray(edge_index).astype(np.int64)
    W1 = np.asarray(W1, dtype=np.float32)
    W2 = np.asarray(W2, dtype=np.float32)
    b1 = np.asarray(b1, dtype=np.float32).reshape(-1)
    b2 = np.asarray(b2, dtype=np.float32).reshape(-1)
    src = ei[:, 0].astype(np.int64)
    dst = ei[:, 1].astype(np.int64)

    caps, per_core = _global_plan(ei)
    plan, EP, DTOT = _plan_sizes(caps)

    # lhsT blocks: [src->h0, src->h1, dst->h0, dst->h1]
    w1_arr = np.concatenate(
        [W1[:D, :D], W1[:D, D:], W1[D:, :D], W1[D:, D:]], axis=1
    ).astype(BF16)
    b1_arr = np.ascontiguousarray(np.stack([b1[:128], b1[128:]], axis=1))
    w2_arr = np.ascontiguousarray(np.stack([W2[:128, 0], W2[128:, 0]], axis=1)).astype(
        BF16
    )
    b2_arr = np.full((128, 1), b2[0], np.float32)
    sel_arr = np.zeros((16, 128), np.float32)
    sel_arr[np.arange(128) % 16, np.arange(128)] = 1.0
    sel_arr = sel_arr.astype(BF16).view(np.int16)

    in_maps = []
    unshard = []
    for c in range(NCORES):
        idx, edge_of_slot = _prepare_core(
            per_core[c], caps, plan, EP, DTOT, src, dst
        )
        unshard.append(edge_of_slot)
        in_maps.append(
            {
                "emd": emd_bf,
                "idx": np.concatenate([sel_arr, idx], axis=1),
                "w1": w1_arr,
                "b1": b1_arr,
                "w2": w2_arr,
                "b2": b2_arr,
            }
        )

    nc = _build_program(caps)
    res = run_bass_kernel_spmd(nc, in_maps, core_ids=list(range(NCORES)))
    LAST_RESULTS = res

    y = np.empty((E_TOTAL,), np.float32)
    for c in range(NCORES):
        edge_of_slot = unshard[c]  # slot -> global edge id
        out = np.asarray(res.results[c]["out"], dtype=np.float32)  # [128, TOT]
        flat = out.T.reshape(-1)  # slot-ordered
        mask = edge_of_slot >= 0
        y[edge_of_slot[mask]] = flat[mask]
    return y.reshape(E_TOTAL, 1)


if __name__ == "__main__":
    rng = np.random.default_rng(0)
    emd = rng.standard_normal((N_NODES, D), dtype=np.float32)
    ei = rng.integers(0, N_NODES, size=(E_TOTAL, 2)).astype(np.int32)
    W1 = rng.standard_normal((2 * D, H), dtype=np.float32) / np.sqrt(2 * D)
    W2 = rng.standard_normal((H, 1), dtype=np.float32) / np.sqrt(H)
    out = kernel(emd, ei, W1, np.zeros(H, np.float32), W2, np.zeros(1, np.float32))
    print(out.shape, out[:4, 0])


# revision 22
# speedup vs baseline: 1.2148x; 1.0018x over previous
"""Link-predictor GNN kernel for 8 TRN2 NeuronCores.

Strategy (per sharding hint): shard edges across 8 cores (data parallel),
replicate the bf16 node-embedding table + MLP weights on every core.

Edges are bucketed by (src_window, dst_window) where a window is 25000
table rows (4 windows cover 100000 nodes) so window-relative node ids fit
the int16 indices of the batched SWDGE dma_gather.

DMA-descriptor reduction via 512B paired descriptors: the SWDGE cost is
per-descriptor with a 2x penalty under 512 bytes, so a 256-element bf16
descriptor (elem_step=128: table rows u and u+1) costs the same as a
128-element one but carries two rows. Per bucket, edges whose SRC rows are
adjacent (u, u+1) are paired globally (path-greedy along each window),
leftovers are paired again by DST adjacency, and the rest stay solo:
~85% of edges land in pairs, cutting gather descriptors per edge from
2.0 to ~1.6. A paired call of n pairs lands [128, 2, n] (sub-block 0 =
first rows, sub-block 1 = second rows, both in X^T layout); the other
side uses a per-slot elem=128 gather. Pairs/solos are dealt round-robin
to cores so the shared static per-bucket capacities are tight.

Each bucket's [SP pairs | solo | DP pairs] column layout is processed as
ONE chunk with 4 merged gather calls (src-pair, src-solo over solo+DP,
dst-solo over SP+solo, dst-pair), keeping the Pool engine's ~1us fixed
SWDGE overhead per call well below the DMA transfer time so descriptor
generation always runs ahead. The first and last buckets are split into
small pieces so the DMA pipeline fills fast and drains short.

MLP per 512-edge tile: h = relu(W1s^T Xs + W1d^T Xd + b1) via 4 matmuls
accumulating in PSUM; relu of h-half-0 on ACT (bias fused), half-1 on DVE
(tensor_scalar add+max). Layer 2 contracts h against W2 using h-subtiles
as the stationary operand: 2 matmuls of N=1 per 128-edge subtile writing
one PSUM column; a whole chunk's logits accumulate into one PSUM tile so a
single sigmoid + one small DMA per chunk emits [128, cols] f32 results.
Host inverts the slot permutation. The 16-partition-wrapped index arrays
are loaded once and replicated to the 128 partitions the gather hardware
expects via 0/1-matmul broadcasts of the raw bf16 bit patterns.
"""

import sys

sys.path.insert(0, "/opt/trn_rl_repo")

import numpy as np
import ml_dtypes

from concourse import bacc, mybir, tile
from concourse.ap import AP
from concourse.bass_utils import run_bass_kernel_spmd

BF16 = ml_dtypes.bfloat16

N_NODES = 100000
D = 128
H = 256
E_TOTAL = 600000
NCORES = 8
WIN = 25000                      # table-row window (< 2^15 for int16 idx)
NBUCK = 16                       # 4 src windows x 4 dst windows
XBUFS = 6                        # gather buffer depth per side
HPBUFS = 2                       # PSUM h depth
L2LAG = 1                        # tiles of lag between L1 and L2 issue
HBUFS = 3                        # h sbuf tile depth
PLBUFS = 2                       # logits PSUM depth
PBBUFS = 2                       # idx-broadcast PSUM depth
OBUFS = 3                        # sigmoid output tile depth
FILL_PIECES = (512,)             # leading col-cuts of the first bucket
TAIL_PIECE = 1024                # trailing bucket split granularity
MIDSPLITS = 2                    # extra column cuts per middle bucket
MIDSPLIT_N, MIDSPLIT_D = 1, 4    # mid output-store point (fraction of plan)

# (sp_pairs, dp_pairs, solo_cols) per bucket for the canonical
# setup_inputs() edge set. kernel() recomputes these from its actual
# inputs; this default only serves _build_program() callers that have no
# inputs (e.g. a standalone TimelineSim of the program).
DEFAULT_CAPS = (
    (1664, 1664, 1664, 1664, 1664, 1664, 1664, 1664,
     1664, 1664, 1664, 1664, 1664, 1664, 1664, 1664),
    (256, 256, 256, 256, 256, 256, 256, 256,
     256, 256, 256, 256, 256, 256, 256, 256),
    (1024, 896, 896, 1024, 896, 896, 896, 896,
     896, 896, 896, 896, 896, 896, 896, 1024),
)  # matches _global_plan(setup_inputs()['edge_index'])

LAST_RESULTS = None
_NC_CACHE: dict = {}


def _window(w):
    base = w * WIN
    return base, min(WIN, N_NODES - base)


def _bucket_cols(caps, b):
    spc, dpc, soc = caps
    return 2 * spc[b] + soc[b] + 2 * dpc[b]


def _calls_for_range(caps, b, lo, hi):
    """Gather calls covering chunk-relative cols [lo, hi) of bucket b's
    [SP | solo | DP] layout. Returns [(side, paired, col_off, ncols)] with
    col_off relative to lo; adjacent same-(side,paired) solo ranges merged.
    Cuts must be 256-aligned so pair calls keep num_idxs % 128 == 0."""
    spc, dpc, soc = caps
    r1 = 2 * spc[b]
    r2 = r1 + soc[b]
    r3 = r2 + 2 * dpc[b]
    assert 0 <= lo < hi <= r3 and lo % 256 == 0 and (hi % 256 == 0 or hi == r3)
    # call order matters for pipelining: [s-pair, d-solo] first makes the SP
    # region (the bulk of the chunk) compute-ready before the solo/DP calls
    # finish, so the MLP tracks the DMA stream with a short lag.
    calls = []
    a, bnd = max(lo, 0), min(hi, r1)
    if a < bnd:
        calls.append(("s", True, a - lo, bnd - a))
    a, bnd = max(lo, 0), min(hi, r2)
    if a < bnd:
        calls.append(("d", False, a - lo, bnd - a))
    a, bnd = max(lo, r1), min(hi, r3)
    if a < bnd:
        calls.append(("s", False, a - lo, bnd - a))
    a, bnd = max(lo, r2), min(hi, r3)
    if a < bnd:
        calls.append(("d", True, a - lo, bnd - a))
    return calls


def _chunk_plan(caps):
    """Ordered chunk list. Chunk = (bucket, lo, cols, slot_base, idx_base,
    calls) where calls = [(side, paired, col_off, ncols, idx_off)], idx_off
    in descriptors relative to idx_base."""
    pieces = []  # (bucket, lo, hi)
    for b in range(NBUCK):
        cols = _bucket_cols(caps, b)
        if cols == 0:
            continue
        if b == 0:
            o = 0
            for fp in FILL_PIECES:
                if o + fp < cols:
                    pieces.append((b, o, o + fp))
                    o += fp
            if o < cols:
                pieces.append((b, o, cols))
        elif b == NBUCK - 1:
            # pieces from the end: ..., 1024, 512, 256 so the drain is short
            cuts = [cols]
            rem = cols
            for tp in (256, 512):
                if rem - tp > 0:
                    rem -= tp
                    cuts.append(rem)
            while rem - TAIL_PIECE > 0:
                rem -= TAIL_PIECE
                cuts.append(rem)
            cuts.append(0)
            cuts = sorted(set(cuts))
            for i in range(len(cuts) - 1):
                pieces.append((b, cuts[i], cuts[i + 1]))
        else:
            cuts = sorted(
                {0, cols}
                | {cols * i // (MIDSPLITS + 1) // 256 * 256
                   for i in range(1, MIDSPLITS + 1)}
            )
            for i in range(len(cuts) - 1):
                pieces.append((b, cuts[i], cuts[i + 1]))
    plan = []
    sbase = 0
    ibase = 0
    for b, lo, hi in pieces:
        calls = []
        ioff = 0
        for side, paired, coff, ncols in _calls_for_range(caps, b, lo, hi):
            nd = ncols // 2 if paired else ncols
            calls.append((side, paired, coff, ncols, ioff))
            ioff += nd
        plan.append((b, lo, hi - lo, sbase, ibase, calls))
        sbase += hi - lo
        ibase += ioff
    return plan


def _plan_sizes(caps):
    plan = _chunk_plan(caps)
    ep = sum(p[2] for p in plan)
    last = plan[-1]
    dtot = last[4] + sum(
        (nc // 2 if paired else nc) for _, paired, _, nc, _ in last[5]
    )
    return plan, ep, dtot


def _build_program(caps=None):
    if caps is None:
        caps = DEFAULT_CAPS
    caps = tuple(tuple(int(x) for x in c) for c in caps)
    if caps in _NC_CACHE:
        return _NC_CACHE[caps]

    dt = mybir.dt
    AF = mybir.ActivationFunctionType
    ALU = mybir.AluOpType

    plan, EP, DTOT = _plan_sizes(caps)
    TOT = EP // 128
    CMAX = max(p[2] for p in plan)
    NDMAX = max(
        sum(nc // 2 if paired else nc for _, paired, _, nc, _ in p[5])
        for p in plan
    )

    nc = bacc.Bacc(
        "TRN2",
        target_bir_lowering=False,
        debug=False,
        enable_asserts=False,
        num_devices=NCORES,
    )
    emd = nc.dram_tensor("emd", [N_NODES, D], dt.bfloat16, kind="ExternalInput")
    idx_d = nc.dram_tensor(
        "idx", [16, 128 + DTOT // 16], dt.int16, kind="ExternalInput"
    )
    w1_d = nc.dram_tensor("w1", [128, 512], dt.bfloat16, kind="ExternalInput")
    b1_d = nc.dram_tensor("b1", [128, 2], dt.float32, kind="ExternalInput")
    w2_d = nc.dram_tensor("w2", [128, 2], dt.bfloat16, kind="ExternalInput")
    b2_d = nc.dram_tensor("b2", [128, 1], dt.float32, kind="ExternalInput")
    out_d = nc.dram_tensor("out", [128, TOT], dt.float32, kind="ExternalOutput")

    with tile.TileContext(nc) as tc:
        with (
            tc.tile_pool(name="const", bufs=1) as cpool,
            tc.tile_pool(name="x", bufs=XBUFS) as xpool,
            tc.tile_pool(name="h", bufs=HBUFS) as hpool,
            tc.tile_pool(name="o", bufs=OBUFS) as opool,
            tc.tile_pool(name="ph", bufs=HPBUFS, space="PSUM") as php,
            tc.tile_pool(name="pl", bufs=PLBUFS, space="PSUM") as plp,
            tc.tile_pool(name="pb", bufs=PBBUFS, space="PSUM") as pbp,
        ):
            # one combined load: the 0/1 selection matrix (bf16-bitcast) in
            # cols 0:128, then all (16-partition-wrapped) gather indices.
            selidx_sb = cpool.tile([16, 128 + DTOT // 16], dt.int16)
            nc.sync.dma_start(selidx_sb[:, :], idx_d[:, :])
            sel_sb = selidx_sb[:, 0:128].bitcast(dt.bfloat16)
            idx16_sb = selidx_sb[:, 128:]
            osb = cpool.tile([128, TOT], dt.float32)
            w1_sb = cpool.tile([128, 512], dt.bfloat16)
            nc.scalar.dma_start(w1_sb[:, :], w1_d[:, :])
            b1_sb = cpool.tile([128, 2], dt.float32)
            nc.scalar.dma_start(b1_sb[:, :], b1_d[:, :])
            w2_sb = cpool.tile([128, 2], dt.bfloat16)
            nc.scalar.dma_start(w2_sb[:, :], w2_d[:, :])
            b2_sb = cpool.tile([128, 1], dt.float32)
            nc.scalar.dma_start(b2_sb[:, :], b2_d[:, :])

            # broadcast every chunk's indices to 128 partitions up front;
            # PE/DVE pipeline stays far ahead of the gathers consuming them
            midcol = 0
            sdis = []
            for k, (b, lo, cols, sbase, ibase, calls) in enumerate(plan):
                nd = sum(c // 2 if p else c for _, p, _, c, _ in calls)
                c16 = ibase // 16
                ibx = pbp.tile([128, NDMAX // 16], dt.float32, tag="ibx")
                nc.tensor.matmul(
                    ibx[:, 0 : nd // 16], lhsT=sel_sb,
                    rhs=idx16_sb[:, c16 : c16 + nd // 16].bitcast(dt.bfloat16),
                    start=True, stop=True,
                )
                sdi = cpool.tile([128, nd // 16], dt.int16, name=f"sdi{k}")
                nc.vector.tensor_copy(
                    out=sdi[:, :].bitcast(dt.bfloat16),
                    in_=ibx[:, 0 : nd // 16],
                )
                sdis.append(sdi)

            for k, (b, lo, cols, sbase, ibase, calls) in enumerate(plan):
                ncols128 = cols // 128
                sb_, sl_ = _window(b >> 2)
                db_, dl_ = _window(b & 3)
                sdi = sdis[k]
                xs = xpool.tile([128, CMAX], dt.bfloat16, tag="xs")
                xd = xpool.tile([128, CMAX], dt.bfloat16, tag="xd")
                for side, paired, coff, cn, ioff in calls:
                    tile_, wb, wl = (
                        (xs, sb_, sl_) if side == "s" else (xd, db_, dl_)
                    )
                    if paired:
                        n = cn // 2
                        pi = sdi[:, ioff // 16 : (ioff + n) // 16]
                        base = emd[wb : wb + wl, :]
                        pview = AP(
                            base.tensor, base.offset,
                            [[128, wl - 1], [1, 256]],
                        )
                        t = tile_[:, coff : coff + cn]
                        out3 = AP(
                            t.tensor, t.offset,
                            [[t.ap[0][0], 128], [n, 2], [1, n]],
                        )
                        nc.gpsimd.dma_gather(
                            out3, pview, pi, n, n, 256,
                            elem_step=128, transpose=True, single_packet=False,
                        )
                    else:
                        si = sdi[:, ioff // 16 : (ioff + cn) // 16]
                        nc.gpsimd.dma_gather(
                            tile_[:, coff : coff + cn].unsqueeze(1),
                            emd[wb : wb + wl, :], si, cn, cn, D,
                            transpose=True, single_packet=False,
                        )

                lg = plp.tile([128, CMAX // 128], dt.float32, tag="lg")
                ntile = (cols + 511) // 512
                # software pipeline: L2 of tile t issues L2LAG tiles late
                hq = []
                for t in range(ntile + L2LAG):
                    if t < ntile:
                        e0 = t * 512
                        n = min(512, cols - e0)
                        h0p = php.tile([128, 512], dt.float32, tag="h0p")
                        h1p = php.tile([128, 512], dt.float32, tag="h1p")
                        nc.tensor.matmul(
                            h0p[:, 0:n], lhsT=w1_sb[:, 0:128],
                            rhs=xs[:, e0 : e0 + n], start=True, stop=False,
                        )
                        nc.tensor.matmul(
                            h0p[:, 0:n], lhsT=w1_sb[:, 256:384],
                            rhs=xd[:, e0 : e0 + n], start=False, stop=True,
                        )
                        nc.tensor.matmul(
                            h1p[:, 0:n], lhsT=w1_sb[:, 128:256],
                            rhs=xs[:, e0 : e0 + n], start=True, stop=False,
                        )
                        nc.tensor.matmul(
                            h1p[:, 0:n], lhsT=w1_sb[:, 384:512],
                            rhs=xd[:, e0 : e0 + n], start=False, stop=True,
                        )
                        h0s = hpool.tile([128, 512], dt.bfloat16, tag="h0s")
                        h1s = hpool.tile([128, 512], dt.bfloat16, tag="h1s")
                        nc.scalar.activation(
                            h0s[:, 0:n], h0p[:, 0:n], AF.Relu, bias=b1_sb[:, 0:1]
                        )
                        nc.vector.tensor_scalar(
                            h1s[:, 0:n], h1p[:, 0:n],
                            b1_sb[:, 1:2], 0.0, ALU.add, ALU.max,
                        )
                        hq.append((t, n, h0s, h1s))
                    if t >= L2LAG:
                        pt, pn, p0, p1 = hq[t - L2LAG]
                        for s in range((pn + 127) // 128):
                            ns = min(128, pn - s * 128)
                            col = pt * 4 + s
                            nc.tensor.matmul(
                                lg[0:ns, col : col + 1],
                                lhsT=p0[:, s * 128 : s * 128 + ns],
                                rhs=w2_sb[:, 0:1], start=True, stop=False,
                            )
                            nc.tensor.matmul(
                                lg[0:ns, col : col + 1],
                                lhsT=p1[:, s * 128 : s * 128 + ns],
                                rhs=w2_sb[:, 1:2], start=False, stop=True,
                            )
                nc.scalar.activation(
                    osb[:, sbase // 128 : sbase // 128 + ncols128],
                    lg[:, 0:ncols128], AF.Sigmoid, bias=b2_sb[:, 0:1],
                )
                if k == len(plan) * MIDSPLIT_N // MIDSPLIT_D and k < len(plan) - 2:
                    midcol = (sbase + cols) // 128
                    nc.sync.dma_start(out_d[:, 0:midcol], osb[:, 0:midcol])
                if k == len(plan) - 2 and midcol < (sbase + cols) // 128:
                    nextcol = (sbase + cols) // 128
                    nc.sync.dma_start(
                        out_d[:, midcol:nextcol], osb[:, midcol:nextcol]
                    )
                    midcol = nextcol
                if k == len(plan) - 1 and midcol < TOT:
                    nc.sync.dma_start(
                        out_d[:, midcol:TOT], osb[:, midcol:TOT]
                    )

    nc.compile()
    _NC_CACHE[caps] = nc
    return nc


def _wrap_idx(vals):
    """int16 [n] -> [16, n//16] wrapped in 16 partitions."""
    n = vals.shape[0]
    return np.ascontiguousarray(vals.reshape(n // 16, 16).T)


def _path_greedy(pos_vals):
    """Pair refs at adjacent window-local positions (p, p+1), each ref used
    once, left-to-right greedy with carry (optimal for paths).

    pos_vals: int array of window-local positions. Returns (a_idx, b_idx):
    indices into pos_vals; ref a at position p pairs with ref b at p+1.
    """
    order = np.argsort(pos_vals, kind="stable")
    pv = pos_vals[order]
    cnt = np.bincount(pv, minlength=WIN)
    starts = np.zeros(WIN + 1, np.int64)
    np.cumsum(cnt, out=starts[1:])
    pairs_a = []
    pairs_b = []
    avail = 0
    prev_p = -2
    for p in np.nonzero(cnt)[0]:
        c = int(cnt[p])
        s = int(starts[p])
        if p == prev_p + 1 and avail > 0:
            t = min(avail, c)
            ps = int(starts[prev_p])
            pc = int(cnt[prev_p])
            pairs_a.append(order[ps + pc - avail : ps + pc - avail + t])
            pairs_b.append(order[s : s + t])
            avail = c - t
        else:
            avail = c
        prev_p = p
    if pairs_a:
        return np.concatenate(pairs_a), np.concatenate(pairs_b)
    return np.empty(0, np.int64), np.empty(0, np.int64)


def _global_plan(ei):
    """Bucket + pair all edges globally, deal to cores.

    Returns (caps, per_core) where caps = (sp[16], dp[16], so[16]) (sp/dp in
    pairs, so in cols) and per_core[c][b] = (sp_pairs [p,2], dp_pairs [p,2],
    solo [s]) holding GLOBAL edge ids."""
    src = ei[:, 0].astype(np.int64)
    dst = ei[:, 1].astype(np.int64)
    bucket = (src // WIN) * 4 + dst // WIN

    spc = [0] * NBUCK
    dpc = [0] * NBUCK
    soc = [0] * NBUCK
    per_core = [dict() for _ in range(NCORES)]
    for b in range(NBUCK):
        sel = np.nonzero(bucket == b)[0]
        a, bb = _path_greedy(src[sel] % WIN)
        paired = np.zeros(len(sel), bool)
        paired[a] = True
        paired[bb] = True
        sp = np.stack([sel[a], sel[bb]], axis=1) if len(a) else \
            np.empty((0, 2), np.int64)
        left = np.nonzero(~paired)[0]
        a2, b2 = _path_greedy(dst[sel[left]] % WIN)
        lp = np.zeros(len(left), bool)
        lp[a2] = True
        lp[b2] = True
        dp = np.stack([sel[left[a2]], sel[left[b2]]], axis=1) if len(a2) else \
            np.empty((0, 2), np.int64)
        so = sel[left[~lp]]

        # pair caps: keep exactly NCORES*cap pairs (cap 128-aligned), demote
        # the rest globally into the solo pool; deal pairs and solos round-
        # robin so every core gets exactly cap pairs and solos within +-1.
        # Solo cap rounds up to 128 cols (256 for the split first/last
        # buckets so region boundaries stay 256-aligned under col cuts).
        nsp = len(sp) // NCORES // 128 * 128
        ndp = len(dp) // NCORES // 128 * 128
        spc[b] = nsp
        dpc[b] = ndp
        keep_sp = sp[: nsp * NCORES]
        keep_dp = dp[: ndp * NCORES]
        solo_pool = np.concatenate(
            [sp[nsp * NCORES :].reshape(-1), dp[ndp * NCORES :].reshape(-1),
             so]
        )
        max_solo = 0
        for c in range(NCORES):
            per_core[c][b] = (
                keep_sp[c::NCORES], keep_dp[c::NCORES], solo_pool[c::NCORES]
            )
            max_solo = max(max_solo, len(solo_pool[c::NCORES]))
        gran = 256 if b in (0, NBUCK - 1) else 128
        soc[b] = -(-max_solo // gran) * gran
    return (tuple(spc), tuple(dpc), tuple(soc)), per_core


def _prepare_core(core_items, caps, plan, EP, DTOT, src, dst):
    """Build one core's wrapped idx array + slot->edge map for the shared
    chunk plan. core_items: dict b -> (sp, dp, solo) global-edge-id arrays."""
    spc, dpc, soc = caps
    idx = np.zeros((16, DTOT // 16), np.int16)
    edge_of_slot = np.full(EP, -1, np.int64)

    # per-bucket col -> edge map (built once per bucket)
    col_edge = {}
    for b in range(NBUCK):
        sp, dp, solo = core_items[b]
        cols = _bucket_cols(caps, b)
        ce = np.full(cols, -1, np.int64)
        r1 = 2 * spc[b]
        r2 = r1 + soc[b]
        # SP region pair p -> cols (2-aligned within each CALL, but the call
        # layout depends on the chunk split; store pairs positionally and let
        # the per-call loop place them)
        col_edge[b] = ce  # filled below per chunk for pair regions
        ce[r1 : r1 + len(solo)] = solo

    for b, lo, cols, sbase, ibase, calls in plan:
        sp, dp, solo = core_items[b]
        sb_ = (b >> 2) * WIN
        db_ = (b & 3) * WIN
        r1 = 2 * spc[b]
        r2 = r1 + soc[b]
        ce = col_edge[b]
        # place pair edges for pair calls of this chunk (sub-block layout is
        # per-call: first rows at cols [coff, coff+n), second rows at
        # [coff+n, coff+2n))
        for side, paired, coff, cn, ioff in calls:
            if not paired:
                continue
            n = cn // 2
            glo = lo + coff
            pairs = sp if side == "s" else dp
            p0 = (glo - (0 if side == "s" else r2)) // 2
            pc_ = pairs[p0 : p0 + n]
            m = len(pc_)
            if m:
                ce[glo : glo + m] = pc_[:, 0]
                ce[glo + n : glo + n + m] = pc_[:, 1]
        # now write idx values for every call
        for side, paired, coff, cn, ioff in calls:
            tab = src if side == "s" else dst
            base_ = sb_ if side == "s" else db_
            if paired:
                n = cn // 2
                e = ce[lo + coff : lo + coff + n]
                v = np.zeros(n, np.int16)
                m = e >= 0
                v[m] = (tab[e[m]] - base_).astype(np.int16)
                idx[:, (ibase + ioff) // 16 : (ibase + ioff + n) // 16] = \
                    _wrap_idx(v)
            else:
                e = ce[lo + coff : lo + coff + cn]
                v = np.zeros(cn, np.int16)
                m = e >= 0
                v[m] = (tab[e[m]] - base_).astype(np.int16)
                idx[:, (ibase + ioff) // 16 : (ibase + ioff + cn) // 16] = \
                    _wrap_idx(v)
        edge_of_slot[sbase : sbase + cols] = ce[lo : lo + cols]
    return idx, edge_of_slot


def kernel(emd_all, edge_index, W1, b1, W2, b2):
    global LAST_RESULTS
    emd_bf = np.ascontiguousarray(np.asarray(emd_all, dtype=np.float32)).astype(BF16)
    ei = np.asarray(edge_index).astype(np.int64)
    W1 = np.asarray(W1, dtype=np.float32)
    W2 = np.asarray(W2, dtype=np.float32)
    b1 = np.asarray(b1, dtype=np.float32).reshape(-1)
    b2 = np.asarray(b2, dtype=np.float32).reshape(-1)
    src = ei[:, 0].astype(np.int64)
    dst = ei[:, 1].astype(np.int64)

    caps, per_core = _global_plan(ei)
    plan, EP, DTOT = _plan_sizes(caps)

    # lhsT blocks: [src->h0, src->h1, dst->h0, dst->h1]
    w1_arr = np.concatenate(
        [W1[:D, :D], W1[:D, D:], W1[D:, :D], W1[D:, D:]], axis=1
    ).astype(BF16)
    b1_arr = np.ascontiguousarray(np.stack([b1[:128], b1[128:]], axis=1))
    w2_arr = np.ascontiguousarray(np.stack([W2[:128, 0], W2[128:, 0]], axis=1)).astype(
        BF16
    )
    b2_arr = np.full((128, 1), b2[0], np.float32)
    sel_arr = np.zeros((16, 128), np.float32)
    sel_arr[np.arange(128) % 16, np.arange(128)] = 1.0
    sel_arr = sel_arr.astype(BF16).view(np.int16)

    in_maps = []
    unshard = []
    for c in range(NCORES):
        idx, edge_of_slot = _prepare_core(
            per_core[c], caps, plan, EP, DTOT, src, dst
        )
        unshard.append(edge_of_slot)
        in_maps.append(
            {
                "emd": emd_bf,
                "idx": np.concatenate([sel_arr, idx], axis=1),
                "w1": w1_arr,
                "b1": b1_arr,
                "w2": w2_arr,
                "b2": b2_arr,
            }
        )

    nc = _build_program(caps)
    res = run_bass_kernel_spmd(nc, in_maps, core_ids=list(range(NCORES)))
    LAST_RESULTS = res

    y = np.empty((E_TOTAL,), np.float32)
    for c in range(NCORES):
        edge_of_slot = unshard[c]  # slot -> global edge id
        out = np.asarray(res.results[c]["out"], dtype=np.float32)  # [128, TOT]
        flat = out.T.reshape(-1)  # slot-ordered
        mask = edge_of_slot >= 0
        y[edge_of_slot[mask]] = flat[mask]
    return y.reshape(E_TOTAL, 1)


if __name__ == "__main__":
    rng = np.random.default_rng(0)
    emd = rng.standard_normal((N_NODES, D), dtype=np.float32)
    ei = rng.integers(0, N_NODES, size=(E_TOTAL, 2)).astype(np.int32)
    W1 = rng.standard_normal((2 * D, H), dtype=np.float32) / np.sqrt(2 * D)
    W2 = rng.standard_normal((H, 1), dtype=np.float32) / np.sqrt(H)
    out = kernel(emd, ei, W1, np.zeros(H, np.float32), W2, np.zeros(1, np.float32))
    print(out.shape, out[:4, 0])
